# revision 1
# baseline (speedup 1.0000x reference)
"""Trainium2 Bass kernel for nn_CondBlock (LayerNorm -> LightGCN conv -> LayerNorm -> 1x1 conv over P).

Self-contained: hardcoded shapes, host-side graph preprocessing, 8-core
data-parallel (over batch) SPMD execution via run_bass_kernel_spmd.

Algorithm (validated vs reference in fp32):
  per slice s=(b,p): LN1: h1 = c_s*(x - mu_s)*g_w + g_b, c_s = rsqrt(var_s+eps)
  conv:  A @ h1 = c_s*(A@(x*g_w)) - (c_s mu_s)*(A@g_w) + A@g_b
         with g_w == const kg folded into A; u = A@g_w, v = A@g_b host consts.
  LN2 + P-mix folded:
         out_q = sum_p aa[q,p]*Z_p + r1[q],  aa[q,p] = conv_w[q,p]*c2_p*kt
         r1[q] = -sum_p aa[q,p]*mu2_p + kb*sum_p conv_w[q,p] + conv_b[q]
  Device: pass-1 matmul  Z^T[(p,h), n] = X'[n,(p,h)]^T @ A^T   (fp32r, X stationary)
          pass-2 matmul  out[n,(q,h)] = Z^T-tiles^T @ W, W = (conv_w (x) I_64)*c2*kt
"""

import numpy as np

B, P, N, H = 16, 12, 2048, 64
E = 16384
NCORES = 8
BL = B // NCORES      # batches per core
PH = P * H            # 768
MC = PH // 128        # 6 (p,h)-chunks of 128
KT = N // 128         # 16 node tiles
FQW = 512             # dst-column chunk width for pass-1
FQ = N // FQW         # 4
NH = float(N * H)
EPS = 1e-5

_CACHE = {}


def _build_program(has_v=False):
    import os
    SKIP = set(filter(None, os.environ.get("K_SKIP", "").split(",")))
    from concourse import bass, bacc, tile, mybir
    from contextlib import ExitStack

    f32 = mybir.dt.float32
    f32r = mybir.dt.float32r
    bf16 = mybir.dt.bfloat16
    ds = bass.ds
    Alu = mybir.AluOpType
    Act = mybir.ActivationFunctionType

    nc = bacc.Bacc("TRN2", target_bir_lowering=False, debug=False)

    x_d = nc.dram_tensor("x", [BL, 128, KT, P, H], bf16, kind="ExternalInput").ap()
    at_d = nc.dram_tensor("at", [N, N], bf16, kind="ExternalInput").ap()
    cwi_d = nc.dram_tensor("cwi", [PH, PH], f32r, kind="ExternalInput").ap()
    ut2_d = nc.dram_tensor("ut2", [128, N], f32, kind="ExternalInput").ap()
    vt2_d = nc.dram_tensor("vt2", [128, N], f32, kind="ExternalInput").ap()
    r12_d = nc.dram_tensor("r12", [P, PH], f32, kind="ExternalInput").ap()
    bo_d = nc.dram_tensor("bo", [PH, P], f32, kind="ExternalInput").ap()
    cwt_d = nc.dram_tensor("cwt", [P, P], f32, kind="ExternalInput").ap()
    cb_d = nc.dram_tensor("cb", [P, 1], f32, kind="ExternalInput").ap()
    out_d = nc.dram_tensor("out", [BL, KT, 128, P, H], f32, kind="ExternalOutput").ap()

    with tile.TileContext(nc) as tc, ExitStack() as ctx:
        cons = ctx.enter_context(tc.tile_pool(name="cons", bufs=1))
        xpool = ctx.enter_context(tc.tile_pool(name="xp", bufs=1))
        zpool = ctx.enter_context(tc.tile_pool(name="zp", bufs=1))
        wpool = ctx.enter_context(tc.tile_pool(name="wp", bufs=1))
        sp = ctx.enter_context(tc.tile_pool(name="sp", bufs=2))
        sml = ctx.enter_context(tc.tile_pool(name="sml", bufs=1))
        pp = ctx.enter_context(tc.tile_pool(name="pp", bufs=6, space="PSUM"))

        # ---- constants ----
        ut2 = cons.tile([128, N], f32, tag="ut2")
        vt2 = cons.tile([128, N], f32, tag="vt2") if has_v else None
        r12 = cons.tile([P, PH], f32, tag="r12")
        bo = cons.tile([128, MC, P], f32, tag="bo")
        cwt = cons.tile([P, P], f32, tag="cwt")
        cb = cons.tile([P, 1], f32, tag="cb")
        onesk = cons.tile([128, 1], bf16, tag="onesk")
        onesm = cons.tile([1, 128], f32, tag="onesm")
        nc.scalar.dma_start(out=ut2[:, :], in_=ut2_d[:, :])
        if has_v:
            nc.scalar.dma_start(out=vt2[:, :], in_=vt2_d[:, :])
        nc.scalar.dma_start(out=r12[:, :], in_=r12_d[:, :])
        nc.scalar.dma_start(out=bo[:, :, :], in_=bo_d.rearrange("(c t) p -> t c p", t=128))
        nc.scalar.dma_start(out=cwt[:, :], in_=cwt_d[:, :])
        nc.scalar.dma_start(out=cb[:, :], in_=cb_d[:, :])
        onesf = cons.tile([128, 1], f32, tag="onesf")
        nc.vector.memset(onesf[:, :], 1.0)
        nc.vector.tensor_copy(onesk[:, :], onesf[:, :])
        nc.vector.memset(onesm[:, :], 1.0)

        atr = ctx.enter_context(tc.tile_pool(name="atr", bufs=1)).tile(
            [128, KT, N], bf16, tag="ATR")

        def load_atr_chunk(kc):
            nc.sync.dma_start(
                out=atr[:, ds(2 * kc, 2), 0:FQW],
                in_=at_d[:, 0:FQW].rearrange("(t k) f -> t k f", k=KT)[:, ds(2 * kc, 2), :])

        def load_atr_rest():
            for fq in range(1, FQ):
                nc.sync.dma_start(
                    out=atr[:, :, ds(fq * FQW, FQW)],
                    in_=at_d[:, ds(fq * FQW, FQW)].rearrange("(t k) f -> t k f", k=KT))

        def mm(out, lhsT, rhs, start, stop):
            nc.tensor.matmul(out, lhsT, rhs, start=start, stop=stop)

        def col12(row):
            """[1,12] sbuf row -> [12,1] sbuf col (via PE)."""
            ps = pp.tile([12, 1], f32, tag="ps")
            mm(ps[:, :], row, onesm[:, 0:1], True, True)
            col = sml.tile([12, 1], f32, tag=None)
            nc.vector.tensor_copy(col[:, :], ps[:, :])
            return col

        def expand12(col_sb, dst):
            """[12,1] sbuf col -> dst [128, MC] per-partition cols (c[p] replicated over h)."""
            for m in range(MC):
                ps = pp.tile([128, 1], f32, tag="ps")
                mm(ps[:, :], r12[:, ds(m * 128, 128)], col_sb, True, True)
                nc.vector.tensor_copy(dst[:, m:m + 1], ps[:, :])

        for b in range(BL):
            # ---- load x (node-major): X[t, k, p, h] = x[b, p, t*16+k, h] ----
            X = xpool.tile([128, KT, P, H], bf16, tag="X")
            for kh in range(8):
                nc.sync.dma_start(
                    out=X[:, ds(2 * kh, 2), :, :],
                    in_=x_d[b][:, ds(2 * kh, 2), :, :])
            if b == 0:
                for kc in range(8):
                    load_atr_chunk(kc)
                load_atr_rest()

            # ---- LN1 stats: PE ones-matmuls, x then x^2 (2 psum banks at a time) ----
            NKS = KT if "stats" not in SKIP else 1
            ps_s1 = pp.tile([1, 2, 512], f32, tag="ps2", name=f"ps_s1_{b}", bufs=1)
            for k in range(NKS):
                for hx in range(2):
                    mm(ps_s1[:, hx, 0:384], onesk[:, :],
                       X[:, k, 6 * hx:6 * hx + 6, :], k == 0, k == NKS - 1)
            s1row = sml.tile([1, PH], f32, tag="s1row")
            for hx in range(2):
                nc.vector.tensor_copy(s1row[:, ds(384 * hx, 384)], ps_s1[:, hx, 0:384])
            ps_q1 = pp.tile([1, 2, 512], f32, tag="ps2", name=f"ps_q1_{b}", bufs=1)
            for k in range(NKS):
                sqx = sp.tile([128, P, H], bf16, tag="sqx")
                nc.scalar.activation(sqx[:, :, :], X[:, k, :, :], Act.Square)
                for hx in range(2):
                    mm(ps_q1[:, hx, 0:384], onesk[:, :],
                       sqx[:, 6 * hx:6 * hx + 6, :], k == 0, k == NKS - 1)
            q1row = sml.tile([1, PH], f32, tag="q1row")
            for hx in range(2):
                nc.vector.tensor_copy(q1row[:, ds(384 * hx, 384)], ps_q1[:, hx, 0:384])
            s1p = sml.tile([1, P], f32, tag="s1p")
            q1p = sml.tile([1, P], f32, tag="q1p")
            with nc.allow_low_precision(reason="12-col reduce in f32"):
                nc.vector.tensor_reduce(s1p[:, :], s1row.rearrange("o (p h) -> o p h", h=H),
                                        mybir.AxisListType.X, Alu.add)
                nc.vector.tensor_reduce(q1p[:, :], q1row.rearrange("o (p h) -> o p h", h=H),
                                        mybir.AxisListType.X, Alu.add)
            s1c = col12(s1p[:, :])
            q1c = col12(q1p[:, :])
            # mu, var, c = rsqrt(var+eps), ncu = -c*mu   (all [12,1])
            mu = sml.tile([P, 1], f32, tag="mu")
            var = sml.tile([P, 1], f32, tag="var")
            tmp = sml.tile([P, 1], f32, tag="tmp")
            c12t = sml.tile([P, 1], f32, tag="c12t")
            ncu12 = sml.tile([P, 1], f32, tag="ncu12")
            nc.vector.tensor_scalar(mu[:, :], s1c[:, :], 1.0 / NH, None, Alu.mult)
            nc.vector.tensor_tensor(tmp[:, :], mu[:, :], mu[:, :], Alu.mult)
            nc.vector.tensor_scalar(var[:, :], q1c[:, :], 1.0 / NH, None, Alu.mult)
            nc.vector.tensor_tensor(var[:, :], var[:, :], tmp[:, :], Alu.subtract)
            nc.vector.tensor_scalar(var[:, :], var[:, :], EPS, None, Alu.add)
            nc.vector.reciprocal(tmp[:, :], var[:, :])
            nc.scalar.activation(c12t[:, :], tmp[:, :], Act.Sqrt)
            nc.vector.scalar_tensor_tensor(ncu12[:, :], c12t[:, :], -1.0, mu[:, :],
                                           Alu.mult, Alu.mult)
            c_col = sml.tile([128, MC], f32, tag="c_col")
            ncu_col = sml.tile([128, MC], f32, tag="ncu_col")
            expand12(c12t[:, :], c_col)
            expand12(ncu12[:, :], ncu_col)

            # ---- W staging: DMA CWI now (scaled by c2 later) ----
            W = wpool.tile([128, MC, PH], f32r, tag="W")
            nc.scalar.dma_start(out=W[:, :, :], in_=cwi_d.rearrange("(c t) f -> t c f", t=128))

            # ---- pass-1 conv: Z^T[(p,h), :] = X^T @ A^T, with LN1 affine on evict ----
            Z = zpool.tile([128, MC, N], f32r, tag="Z")
            zs_slots = sml.tile([128, MC, FQ], f32, tag="zs")
            zq_slots = sml.tile([128, MC, FQ], f32, tag="zq")
            for fq in range(FQ):
                gps = [pp.tile([128, FQW], f32, tag="ps", name=f"gps_{b}_{fq}_{i}") for i in range(MC)]
                NKC = KT if "conv" not in SKIP else 1
                if fq == 0:
                    for k in range(NKC):
                        for m in range(MC):
                            nc.tensor.matmul(gps[m][:, :], X[:, k, 2 * m:2 * m + 2, :],
                                             atr[:, k, ds(fq * FQW, FQW)],
                                             start=k == 0, stop=k == NKC - 1)
                else:
                    for m in range(MC):
                        for k in range(NKC):
                            nc.tensor.matmul(gps[m][:, :], X[:, k, 2 * m:2 * m + 2, :],
                                             atr[:, k, ds(fq * FQW, FQW)],
                                             start=k == 0, stop=k == NKC - 1)
                for m in range(MC if "evict" not in SKIP else 0):
                    corr = sp.tile([128, FQW], f32, tag="corr")
                    if has_v:
                        nc.vector.scalar_tensor_tensor(
                            corr[:, :], ut2[:, ds(fq * FQW, FQW)], ncu_col[:, m:m + 1],
                            vt2[:, ds(fq * FQW, FQW)], Alu.mult, Alu.add)
                    else:
                        nc.vector.tensor_scalar(corr[:, :], ut2[:, ds(fq * FQW, FQW)],
                                                ncu_col[:, m:m + 1], None, Alu.mult)
                    nc.vector.scalar_tensor_tensor(
                        Z[:, m, ds(fq * FQW, FQW)], gps[m][:, :], c_col[:, m:m + 1],
                        corr[:, :], Alu.mult, Alu.add,
                        accum_out=zs_slots[:, m, fq:fq + 1])
                    sqz = sp.tile([128, FQW], f32, tag="sqz")
                    nc.scalar.activation(sqz[:, :], Z[:, m, ds(fq * FQW, FQW)],
                                         Act.Square, accum_out=zq_slots[:, m, fq:fq + 1])

            # ---- LN2 stats ----
            zs6 = sml.tile([128, MC], f32, tag="zs6")
            zq6 = sml.tile([128, MC], f32, tag="zq6")
            with nc.allow_low_precision(reason="f32r == f32 bits; 4-col reduce"):
                nc.vector.tensor_reduce(zs6[:, :], zs_slots[:, :, :], mybir.AxisListType.X, Alu.add)
                nc.vector.tensor_reduce(zq6[:, :], zq_slots[:, :, :], mybir.AxisListType.X, Alu.add)
            ps_s2 = pp.tile([P, 1], f32, tag="ps")
            ps_q2 = pp.tile([P, 1], f32, tag="ps")
            for m in range(MC):
                mm(ps_s2[:, :], bo[:, m, :], zs6[:, m:m + 1], m == 0, m == MC - 1)
                mm(ps_q2[:, :], bo[:, m, :], zq6[:, m:m + 1], m == 0, m == MC - 1)
            s2c = sml.tile([P, 1], f32, tag="s2c")
            q2c = sml.tile([P, 1], f32, tag="q2c")
            nc.vector.tensor_copy(s2c[:, :], ps_s2[:, :])
            nc.vector.tensor_copy(q2c[:, :], ps_q2[:, :])
            mu2 = sml.tile([P, 1], f32, tag="mu2")
            var2 = sml.tile([P, 1], f32, tag="var2")
            tmp2 = sml.tile([P, 1], f32, tag="tmp2")
            c2t = sml.tile([P, 1], f32, tag="c2t")
            nc.vector.tensor_scalar(mu2[:, :], s2c[:, :], 1.0 / NH, None, Alu.mult)
            nc.vector.tensor_tensor(tmp2[:, :], mu2[:, :], mu2[:, :], Alu.mult)
            nc.vector.tensor_scalar(var2[:, :], q2c[:, :], 1.0 / NH, None, Alu.mult)
            nc.vector.tensor_tensor(var2[:, :], var2[:, :], tmp2[:, :], Alu.subtract)
            nc.vector.tensor_scalar(var2[:, :], var2[:, :], EPS, None, Alu.add)
            nc.vector.reciprocal(tmp2[:, :], var2[:, :])
            nc.scalar.activation(c2t[:, :], tmp2[:, :], Act.Sqrt)
            c2_col = sml.tile([128, MC], f32, tag="c2col")
            expand12(c2t[:, :], c2_col)
            # W = CWI * c2 (per-partition scale)
            for m in range(MC):
                nc.vector.tensor_scalar(W[:, m, :], W[:, m, :], c2_col[:, m:m + 1],
                                        None, Alu.mult)
            def emit_r1():
                # r1[q] = cb[q] - sum_p A1[p,q]*mu2[p],  A1 = cwt*c2
                a1 = sml.tile([P, P], f32, tag="a1")
                nc.vector.tensor_scalar(a1[:, :], cwt[:, :], c2t[:, :], None, Alu.mult)
                ps_k1 = pp.tile([P, 1], f32, tag="ps2", bufs=1, name="ps_k1_r1")
                mm(ps_k1[:, :], a1[:, :], mu2[:, :], True, True)
                r1c = sml.tile([P, 1], f32, tag="r1c")
                nc.vector.tensor_tensor(r1c[:, :], cb[:, :], ps_k1[:, :], Alu.subtract)
                r1row = sml.tile([1, PH], f32, tag="r1row")
                r1B = sml.tile([128, PH], f32, tag="r1B")
                for hx in range(2):
                    psr = pp.tile([1, 384], f32, tag="ps2", bufs=1, name=f"psr_{hx}")
                    mm(psr[:, :], r1c[:, :], r12[:, ds(384 * hx, 384)], True, True)
                    nc.vector.tensor_copy(r1row[:, ds(384 * hx, 384)], psr[:, :])
                for hx in range(2):
                    psb = pp.tile([128, 384], f32, tag="ps2", bufs=1, name=f"psb_{hx}")
                    mm(psb[:, :], onesm[:, :], r1row[:, ds(384 * hx, 384)], True, True)
                    nc.vector.tensor_copy(r1B[:, ds(384 * hx, 384)], psb[:, :])
                return r1B

            # ---- pass-2: out[n, (q,h)] = sum_c Z[:, c, n]^T @ W[:, c, :] ----
            r1B = None
            for ni in range(KT):
                po = [pp.tile([128, 384], f32, tag="ps", name=f"po_{b}_{ni}_{i}") for i in range(2)]
                for kc in range(MC if "pass2" not in SKIP else 1):
                    for hx in range(2):
                        mm(po[hx][:, :], Z[:, kc, ds(ni * 128, 128)],
                           W[:, kc, ds(384 * hx, 384)], kc == 0,
                           (kc == MC - 1 or "pass2" in SKIP))
                if r1B is None:
                    r1B = emit_r1()
                if ni % 2 == 0:
                    stage4 = sp.tile([128, 2, P, H], f32, tag="ostage")
                for hx in range(2):
                    nc.vector.tensor_tensor(
                        stage4[:, ni % 2, ds(6 * hx, 6), :],
                        po[hx].rearrange("t (p h) -> t p h", h=H),
                        r1B[:, ds(384 * hx, 384)].rearrange("t (p h) -> t p h", h=H),
                        Alu.add)
                if "out" not in SKIP and ni >= KT - 2:
                    eng = nc.scalar if ni % 2 == 0 else nc.gpsimd
                    eng.dma_start(
                        out=out_d[b][ni, :, :, :],
                        in_=stage4[:, ni % 2, :, :])
                elif "out" not in SKIP and ni % 2 == 1:
                    eng = nc.scalar if (ni // 2) % 2 == 0 else nc.gpsimd
                    eng.dma_start(
                        out=out_d[b][ds(ni - 1, 2), :, :, :].transpose([1, 0, 2, 3]),
                        in_=stage4[:, :, :, :])

    nc.compile()
    return nc


def _host_prep(inputs):
    import ml_dtypes
    x = np.asarray(inputs["x"], dtype=np.float32).astype(ml_dtypes.bfloat16)
    # device layout: [b, t, k, p, h] with node n = t*16 + k
    x = np.ascontiguousarray(x.reshape(B, P, 128, KT, H).transpose(0, 2, 3, 1, 4))
    edge_index = np.asarray(inputs["edge_index"])
    g_w = np.asarray(inputs["g_norm_w"], dtype=np.float32)
    g_b = np.asarray(inputs["g_norm_b"], dtype=np.float32)
    t_w = np.asarray(inputs["t_norm_w"], dtype=np.float32)
    t_b = np.asarray(inputs["t_norm_b"], dtype=np.float32)
    conv_w = np.asarray(inputs["conv_w"], dtype=np.float32)
    conv_b = np.asarray(inputs["conv_b"], dtype=np.float32)

    # fast path requires LN affine params constant (true for this problem family)
    assert np.all(g_w == g_w.flat[0]) and np.all(t_w == t_w.flat[0]), \
        "non-constant LayerNorm weight not supported by this kernel"
    kg = float(g_w.flat[0])
    kt = float(t_w.flat[0])
    assert np.all(t_b == t_b.flat[0]), "non-constant t_norm_b not supported"
    kb = float(t_b.flat[0])

    src = edge_index[0].astype(np.int64)
    dst = edge_index[1].astype(np.int64)
    deg = np.zeros(N, np.float32)
    np.add.at(deg, dst, np.float32(1.0))
    with np.errstate(divide="ignore"):
        dinv = np.where(deg > 0, 1.0 / np.sqrt(np.maximum(deg, 1.0)), 0.0).astype(np.float32)
    norm = dinv[src] * dinv[dst]
    A = np.zeros((N, N), np.float32)
    np.add.at(A, (dst, src), norm)

    u = A @ g_w          # [N, H]
    v = A @ g_b          # [N, H]
    AT = np.ascontiguousarray((A * kg).T)

    ut2 = np.empty((128, N), np.float32)
    vt2 = np.empty((128, N), np.float32)
    ut2[:64] = u.T; ut2[64:] = u.T
    vt2[:64] = v.T; vt2[64:] = v.T

    cwi = np.zeros((PH, PH), np.float32)
    for p in range(P):
        for q in range(P):
            w = conv_w[q, p] * kt
            idx = np.arange(H)
            cwi[p * H + idx, q * H + idx] = w

    r12 = np.zeros((P, PH), np.float32)
    for p in range(P):
        r12[p, p * H:(p + 1) * H] = 1.0
    bo = np.zeros((PH, P), np.float32)
    for p in range(P):
        bo[p * H:(p + 1) * H, p] = 1.0
    cwt = np.ascontiguousarray(conv_w.T * kt)
    cb = (conv_b + kb * conv_w.sum(axis=1)).astype(np.float32).reshape(P, 1)

    import ml_dtypes
    AT = AT.astype(ml_dtypes.bfloat16)
    consts = {"at": AT, "cwi": cwi, "ut2": ut2, "vt2": vt2,
              "r12": r12, "bo": bo, "cwt": cwt, "cb": cb}
    has_v = bool(np.any(v != 0))
    return x, consts, has_v


def _unpack_out(arr):
    """[BL, KT(ni), 128, P, H] -> [BL, P, N, H] with n = ni*128 + t."""
    return np.ascontiguousarray(arr.transpose(0, 3, 1, 2, 4).reshape(BL, P, N, H))


def kernel(**inputs):
    from concourse.bass_utils import run_bass_kernel_spmd

    x, consts, has_v = _host_prep(inputs)

    if ("nc", has_v) not in _CACHE:
        _CACHE[("nc", has_v)] = _build_program(has_v)
    nc = _CACHE[("nc", has_v)]

    in_maps = []
    for c in range(NCORES):
        m = {"x": np.ascontiguousarray(x[c * BL:(c + 1) * BL])}
        m.update(consts)
        in_maps.append(m)

    res = run_bass_kernel_spmd(nc, in_maps, core_ids=list(range(NCORES)))
    out = np.empty((B, P, N, H), np.float32)
    for c in range(NCORES):
        out[c * BL:(c + 1) * BL] = _unpack_out(res.results[c]["out"])
    return out



# revision 11
# speedup vs baseline: 1.1686x; 1.1686x over previous
"""Trainium2 Bass kernel for nn_CondBlock (LayerNorm -> LightGCN conv -> LayerNorm -> 1x1 conv over P).

Self-contained: hardcoded shapes, host-side graph preprocessing, 8-core
data-parallel (over batch) SPMD execution via run_bass_kernel_spmd.

Algorithm (validated vs reference in fp32):
  per slice s=(b,p): LN1: h1 = c_s*(x - mu_s)*g_w + g_b, c_s = rsqrt(var_s+eps)
  conv:  A = D_dst S D_src with S integer edge counts (exact in fp8).
         Device matmul computes G = S^T @ (dinv_src * x) using fp8e4
         DoubleRow matmuls (hi+lo residual split of the scaled x, both
         accumulated in PSUM -> ~1e-3 precision at 2x bf16 PE rate).
         Evict: Z = (c*G + ncu*u1 [+ v1]) * dd, dd = kg*dinv_dst (per col).
  LN2 + P-mix folded:
         out_q = sum_p aa[q,p]*Z_p + r1[q],  aa[q,p] = conv_w[q,p]*c2_p*kt
         r1[q] = -sum_p aa[q,p]*mu2_p + kb*sum_p conv_w[q,p] + conv_b[q]
  Pass-2 matmul  out[n,(q,h)] = Z^T-tiles^T @ W (bf16), W = (conv_w (x) I_64)*c2*kt
"""

import numpy as np

B, P, N, H = 16, 12, 2048, 64
E = 16384
NCORES = 8
BL = B // NCORES      # batches per core
PH = P * H            # 768
MC = PH // 128        # 6 (p,h)-chunks of 128
KT = N // 128         # 16 node tiles
KP = 8                # DoubleRow src chunk pairs (256 nodes each)
FQW = 512             # dst-column chunk width for pass-1
FQ = N // FQW         # 4
NH = float(N * H)
EPS = 1e-5

_CACHE = {}


def _build_program(has_v=False):
    import os
    SKIP = set(filter(None, os.environ.get("K_SKIP", "").split(",")))
    from concourse import bass, bacc, tile, mybir
    from contextlib import ExitStack

    f32 = mybir.dt.float32
    bf16 = mybir.dt.bfloat16
    fp8 = mybir.dt.float8e4
    ds = bass.ds
    Alu = mybir.AluOpType
    Act = mybir.ActivationFunctionType
    DR = mybir.MatmulPerfMode.DoubleRow

    nc = bacc.Bacc("TRN2", target_bir_lowering=False, debug=False)

    xb_d = nc.dram_tensor("xb", [BL, 128, KT, P, H], bf16, kind="ExternalInput").ap()
    x8h_d = nc.dram_tensor("x8h", [BL, 128, KP, 2, P, H], fp8, kind="ExternalInput").ap()
    x8l_d = nc.dram_tensor("x8l", [BL, 128, KP, 2, P, H], fp8, kind="ExternalInput").ap()
    s8_d = nc.dram_tensor("s8", [128, KP, 2, N], fp8, kind="ExternalInput").ap()
    u1_d = nc.dram_tensor("u1", [128, N], f32, kind="ExternalInput").ap()
    dd_d = nc.dram_tensor("dd", [128, N], f32, kind="ExternalInput").ap()
    v1_d = nc.dram_tensor("v1", [128, N], f32, kind="ExternalInput").ap() if has_v else None
    cwi_d = nc.dram_tensor("cwi", [PH, PH], bf16, kind="ExternalInput").ap()
    r12_d = nc.dram_tensor("r12", [P, PH], f32, kind="ExternalInput").ap()
    bo_d = nc.dram_tensor("bo", [PH, P], f32, kind="ExternalInput").ap()
    cwt_d = nc.dram_tensor("cwt", [P, P], f32, kind="ExternalInput").ap()
    cb_d = nc.dram_tensor("cb", [P, 1], f32, kind="ExternalInput").ap()
    out_d = nc.dram_tensor("out", [BL, KT, 128, P, H], f32, kind="ExternalOutput").ap()

    with tile.TileContext(nc) as tc, ExitStack() as ctx:
        cons = ctx.enter_context(tc.tile_pool(name="cons", bufs=1))
        xpool = ctx.enter_context(tc.tile_pool(name="xp", bufs=1))
        zpool = ctx.enter_context(tc.tile_pool(name="zp", bufs=1))
        wpool = ctx.enter_context(tc.tile_pool(name="wp", bufs=1))
        sp = ctx.enter_context(tc.tile_pool(name="sp", bufs=2))
        sml = ctx.enter_context(tc.tile_pool(name="sml", bufs=1))
        pp = ctx.enter_context(tc.tile_pool(name="pp", bufs=6, space="PSUM"))

        # ---- constants ----
        u1t = cons.tile([128, N], f32, tag="u1t")
        ddt = cons.tile([128, N], f32, tag="ddt")
        v1t = cons.tile([128, N], f32, tag="v1t") if has_v else None
        r12 = cons.tile([P, PH], f32, tag="r12")
        bo = cons.tile([128, MC, P], f32, tag="bo")
        cwt = cons.tile([P, P], f32, tag="cwt")
        cb = cons.tile([P, 1], f32, tag="cb")
        onesk = cons.tile([128, 1], bf16, tag="onesk")
        onesm = cons.tile([1, 128], f32, tag="onesm")
        nc.scalar.dma_start(out=u1t[:, :], in_=u1_d[:, :])
        nc.scalar.dma_start(out=ddt[:, :], in_=dd_d[:, :])
        if has_v:
            nc.scalar.dma_start(out=v1t[:, :], in_=v1_d[:, :])
        nc.scalar.dma_start(out=r12[:, :], in_=r12_d[:, :])
        nc.scalar.dma_start(out=bo[:, :, :], in_=bo_d.rearrange("(c t) p -> t c p", t=128))
        nc.scalar.dma_start(out=cwt[:, :], in_=cwt_d[:, :])
        nc.scalar.dma_start(out=cb[:, :], in_=cb_d[:, :])
        onesf = cons.tile([128, 1], f32, tag="onesf")
        nc.vector.memset(onesf[:, :], 1.0)
        nc.vector.tensor_copy(onesk[:, :], onesf[:, :])
        nc.vector.memset(onesm[:, :], 1.0)

        s8 = ctx.enter_context(tc.tile_pool(name="s8p", bufs=1)).tile(
            [128, KP, 2, N], fp8, tag="S8")

        def load_s8():
            for kc in range(4):
                nc.sync.dma_start(
                    out=s8[:, ds(2 * kc, 2), :, :],
                    in_=s8_d[:, ds(2 * kc, 2), :, :])

        def mm(out, lhsT, rhs, start, stop):
            nc.tensor.matmul(out, lhsT, rhs, start=start, stop=stop)

        def col12(row):
            """[1,12] sbuf row -> [12,1] sbuf col (via PE)."""
            ps = pp.tile([12, 1], f32, tag="ps")
            mm(ps[:, :], row, onesm[:, 0:1], True, True)
            col = sml.tile([12, 1], f32, tag=None)
            nc.vector.tensor_copy(col[:, :], ps[:, :])
            return col

        def expand12(col_sb, dst):
            """[12,1] sbuf col -> dst [128, MC] per-partition cols (c[p] replicated over h)."""
            for m in range(MC):
                ps = pp.tile([128, 1], f32, tag="ps")
                mm(ps[:, :], r12[:, ds(m * 128, 128)], col_sb, True, True)
                nc.vector.tensor_copy(dst[:, m:m + 1], ps[:, :])

        for b in range(BL):
            # ---- load x: bf16 node-major for stats + fp8 hi/lo for conv ----
            X = xpool.tile([128, KT, P, H], bf16, tag="X")
            for kh in range(4):
                nc.sync.dma_start(
                    out=X[:, ds(4 * kh, 4), :, :],
                    in_=xb_d[b][:, ds(4 * kh, 4), :, :])
            X8h = xpool.tile([128, KP, 2, P, H], fp8, tag="X8h")
            X8l = xpool.tile([128, KP, 2, P, H], fp8, tag="X8l")
            for kh in range(2):
                nc.gpsimd.dma_start(
                    out=X8h[:, ds(4 * kh, 4), :, :, :],
                    in_=x8h_d[b][:, ds(4 * kh, 4), :, :, :])
                nc.gpsimd.dma_start(
                    out=X8l[:, ds(4 * kh, 4), :, :, :],
                    in_=x8l_d[b][:, ds(4 * kh, 4), :, :, :])
            if b == 0:
                load_s8()

            # ---- LN1 stats: PE ones-matmuls, x then x^2 (2 psum banks at a time) ----
            NKS = KT if "stats" not in SKIP else 1
            ps_s1 = pp.tile([1, 2, 512], f32, tag="ps2", name=f"ps_s1_{b}", bufs=1)
            for k in range(NKS):
                for hx in range(2):
                    mm(ps_s1[:, hx, 0:384], onesk[:, :],
                       X[:, k, 6 * hx:6 * hx + 6, :], k == 0, k == NKS - 1)
            s1row = sml.tile([1, PH], f32, tag="s1row")
            for hx in range(2):
                nc.vector.tensor_copy(s1row[:, ds(384 * hx, 384)], ps_s1[:, hx, 0:384])
            ps_q1 = pp.tile([1, 2, 512], f32, tag="ps2", name=f"ps_q1_{b}", bufs=1)
            for k in range(NKS):
                sqx = sp.tile([128, P, H], bf16, tag="sqx")
                nc.scalar.activation(sqx[:, :, :], X[:, k, :, :], Act.Square)
                for hx in range(2):
                    mm(ps_q1[:, hx, 0:384], onesk[:, :],
                       sqx[:, 6 * hx:6 * hx + 6, :], k == 0, k == NKS - 1)
            q1row = sml.tile([1, PH], f32, tag="q1row")
            for hx in range(2):
                nc.vector.tensor_copy(q1row[:, ds(384 * hx, 384)], ps_q1[:, hx, 0:384])
            s1p = sml.tile([1, P], f32, tag="s1p")
            q1p = sml.tile([1, P], f32, tag="q1p")
            with nc.allow_low_precision(reason="12-col reduce in f32"):
                nc.vector.tensor_reduce(s1p[:, :], s1row.rearrange("o (p h) -> o p h", h=H),
                                        mybir.AxisListType.X, Alu.add)
                nc.vector.tensor_reduce(q1p[:, :], q1row.rearrange("o (p h) -> o p h", h=H),
                                        mybir.AxisListType.X, Alu.add)
            s1c = col12(s1p[:, :])
            q1c = col12(q1p[:, :])
            # mu, var, c = rsqrt(var+eps), ncu = -c*mu   (all [12,1])
            mu = sml.tile([P, 1], f32, tag="mu")
            var = sml.tile([P, 1], f32, tag="var")
            tmp = sml.tile([P, 1], f32, tag="tmp")
            c12t = sml.tile([P, 1], f32, tag="c12t")
            ncu12 = sml.tile([P, 1], f32, tag="ncu12")
            nc.vector.tensor_scalar(mu[:, :], s1c[:, :], 1.0 / NH, None, Alu.mult)
            nc.vector.tensor_tensor(tmp[:, :], mu[:, :], mu[:, :], Alu.mult)
            nc.vector.tensor_scalar(var[:, :], q1c[:, :], 1.0 / NH, None, Alu.mult)
            nc.vector.tensor_tensor(var[:, :], var[:, :], tmp[:, :], Alu.subtract)
            nc.vector.tensor_scalar(var[:, :], var[:, :], EPS, None, Alu.add)
            nc.vector.reciprocal(tmp[:, :], var[:, :])
            nc.scalar.activation(c12t[:, :], tmp[:, :], Act.Sqrt)
            nc.vector.scalar_tensor_tensor(ncu12[:, :], c12t[:, :], -1.0, mu[:, :],
                                           Alu.mult, Alu.mult)
            c_col = sml.tile([128, MC], f32, tag="c_col")
            ncu_col = sml.tile([128, MC], f32, tag="ncu_col")
            expand12(c12t[:, :], c_col)
            expand12(ncu12[:, :], ncu_col)

            # ---- W staging: DMA CWI now (scaled by c2 later) ----
            W = wpool.tile([128, MC, PH], bf16, tag="W")
            nc.scalar.dma_start(out=W[:, :, :], in_=cwi_d.rearrange("(c t) f -> t c f", t=128))

            # ---- pass-1 conv: G = S^T @ (dinv_src*x) via fp8 DoubleRow hi+lo ----
            Z = zpool.tile([128, MC, N], bf16, tag="Z")
            zs_slots = sml.tile([128, MC, FQ], f32, tag="zs")
            zq_slots = sml.tile([128, MC, FQ], f32, tag="zq")
            for fq in range(FQ):
                gps = [pp.tile([128, FQW], f32, tag="ps", name=f"gps_{b}_{fq}_{i}") for i in range(MC)]
                NKC = KP if "conv" not in SKIP else 1
                for kp in range(NKC):
                    for si, Xs in enumerate((X8h, X8l)):
                        for m in range(MC):
                            nc.tensor.matmul(
                                gps[m][:, :], Xs[:, kp, :, 2 * m:2 * m + 2, :],
                                s8[:, kp, :, ds(fq * FQW, FQW)],
                                start=(kp == 0 and si == 0),
                                stop=(kp == NKC - 1 and si == 1),
                                perf_mode=DR)
                for m in range(MC if "evict" not in SKIP else 0):
                    corr = sp.tile([128, FQW], f32, tag="corr")
                    if has_v:
                        nc.vector.scalar_tensor_tensor(
                            corr[:, :], u1t[:, ds(fq * FQW, FQW)], ncu_col[:, m:m + 1],
                            v1t[:, ds(fq * FQW, FQW)], Alu.mult, Alu.add)
                    else:
                        nc.scalar.activation(corr[:, :], u1t[:, ds(fq * FQW, FQW)],
                                             Act.Copy, scale=ncu_col[:, m:m + 1])
                    zt = sp.tile([128, FQW], f32, tag="zt")
                    nc.vector.scalar_tensor_tensor(
                        zt[:, :], gps[m][:, :], c_col[:, m:m + 1],
                        corr[:, :], Alu.mult, Alu.add)
                    with nc.allow_low_precision(reason="Z stored bf16 for pass-2"):
                        nc.vector.scalar_tensor_tensor(
                            Z[:, m, ds(fq * FQW, FQW)], zt[:, :], 1.0,
                            ddt[:, ds(fq * FQW, FQW)], Alu.mult, Alu.mult,
                            accum_out=zs_slots[:, m, fq:fq + 1])
                    sqz = sp.tile([128, FQW], f32, tag="sqz")
                    zsl = Z[:, m, ds(fq * FQW, FQW)]
                    if m % 2 == 0:
                        nc.scalar.activation(sqz[:, :], zsl, Act.Square,
                                             accum_out=zq_slots[:, m, fq:fq + 1])
                    else:
                        nc.vector.scalar_tensor_tensor(sqz[:, :], zsl, 1.0, zsl,
                                                       Alu.mult, Alu.mult,
                                                       accum_out=zq_slots[:, m, fq:fq + 1])

            # ---- LN2 stats ----
            zs6 = sml.tile([128, MC], f32, tag="zs6")
            zq6 = sml.tile([128, MC], f32, tag="zq6")
            with nc.allow_low_precision(reason="4-col reduce in f32"):
                nc.vector.tensor_reduce(zs6[:, :], zs_slots[:, :, :], mybir.AxisListType.X, Alu.add)
                nc.vector.tensor_reduce(zq6[:, :], zq_slots[:, :, :], mybir.AxisListType.X, Alu.add)
            ps_s2 = pp.tile([P, 1], f32, tag="ps")
            ps_q2 = pp.tile([P, 1], f32, tag="ps")
            for m in range(MC):
                mm(ps_s2[:, :], bo[:, m, :], zs6[:, m:m + 1], m == 0, m == MC - 1)
                mm(ps_q2[:, :], bo[:, m, :], zq6[:, m:m + 1], m == 0, m == MC - 1)
            s2c = sml.tile([P, 1], f32, tag="s2c")
            q2c = sml.tile([P, 1], f32, tag="q2c")
            nc.vector.tensor_copy(s2c[:, :], ps_s2[:, :])
            nc.vector.tensor_copy(q2c[:, :], ps_q2[:, :])
            mu2 = sml.tile([P, 1], f32, tag="mu2")
            var2 = sml.tile([P, 1], f32, tag="var2")
            tmp2 = sml.tile([P, 1], f32, tag="tmp2")
            c2t = sml.tile([P, 1], f32, tag="c2t")
            nc.vector.tensor_scalar(mu2[:, :], s2c[:, :], 1.0 / NH, None, Alu.mult)
            nc.vector.tensor_tensor(tmp2[:, :], mu2[:, :], mu2[:, :], Alu.mult)
            nc.vector.tensor_scalar(var2[:, :], q2c[:, :], 1.0 / NH, None, Alu.mult)
            nc.vector.tensor_tensor(var2[:, :], var2[:, :], tmp2[:, :], Alu.subtract)
            nc.vector.tensor_scalar(var2[:, :], var2[:, :], EPS, None, Alu.add)
            nc.vector.reciprocal(tmp2[:, :], var2[:, :])
            nc.scalar.activation(c2t[:, :], tmp2[:, :], Act.Sqrt)
            c2_col = sml.tile([128, MC], f32, tag="c2col")
            expand12(c2t[:, :], c2_col)
            # W = CWI * c2 (per-partition scale)
            with nc.allow_low_precision(reason="W bf16"):
                for m in range(MC):
                    nc.gpsimd.tensor_scalar(W[:, m, :], W[:, m, :], c2_col[:, m:m + 1],
                                            None, Alu.mult)

            def emit_r1():
                # r1[q] = cb[q] - sum_p A1[p,q]*mu2[p],  A1 = cwt*c2
                a1 = sml.tile([P, P], f32, tag="a1")
                nc.vector.tensor_scalar(a1[:, :], cwt[:, :], c2t[:, :], None, Alu.mult)
                ps_k1 = pp.tile([P, 1], f32, tag="ps2", bufs=1, name="ps_k1_r1")
                mm(ps_k1[:, :], a1[:, :], mu2[:, :], True, True)
                r1c = sml.tile([P, 1], f32, tag="r1c")
                nc.vector.tensor_tensor(r1c[:, :], cb[:, :], ps_k1[:, :], Alu.subtract)
                r1row = sml.tile([1, PH], f32, tag="r1row")
                r1B = sml.tile([128, PH], f32, tag="r1B")
                for hx in range(2):
                    psr = pp.tile([1, 384], f32, tag="ps2", bufs=1, name=f"psr_{hx}")
                    mm(psr[:, :], r1c[:, :], r12[:, ds(384 * hx, 384)], True, True)
                    nc.vector.tensor_copy(r1row[:, ds(384 * hx, 384)], psr[:, :])
                for hx in range(2):
                    psb = pp.tile([128, 384], f32, tag="ps2", bufs=1, name=f"psb_{hx}")
                    mm(psb[:, :], onesm[:, :], r1row[:, ds(384 * hx, 384)], True, True)
                    nc.vector.tensor_copy(r1B[:, ds(384 * hx, 384)], psb[:, :])
                return r1B

            # ---- pass-2: out[n, (q,h)] = sum_c Z[:, c, n]^T @ W[:, c, :] ----
            r1B = None
            for ni in range(KT):
                po = [pp.tile([128, 384], f32, tag="ps", name=f"po_{b}_{ni}_{i}") for i in range(2)]
                for kc in range(MC if "pass2" not in SKIP else 1):
                    for hx in range(2):
                        mm(po[hx][:, :], Z[:, kc, ds(ni * 128, 128)],
                           W[:, kc, ds(384 * hx, 384)], kc == 0,
                           (kc == MC - 1 or "pass2" in SKIP))
                if r1B is None:
                    r1B = emit_r1()
                if ni % 2 == 0:
                    stage4 = sp.tile([128, 2, P, H], f32, tag="ostage")
                for hx in range(2):
                    nc.vector.tensor_tensor(
                        stage4[:, ni % 2, ds(6 * hx, 6), :],
                        po[hx].rearrange("t (p h) -> t p h", h=H),
                        r1B[:, ds(384 * hx, 384)].rearrange("t (p h) -> t p h", h=H),
                        Alu.add)
                if "out" not in SKIP and ni >= KT - 2:
                    eng = nc.scalar if ni % 2 == 0 else nc.gpsimd
                    eng.dma_start(
                        out=out_d[b][ni, :, :, :],
                        in_=stage4[:, ni % 2, :, :])
                elif "out" not in SKIP and ni % 2 == 1:
                    eng = nc.scalar if (ni // 2) % 2 == 0 else nc.gpsimd
                    eng.dma_start(
                        out=out_d[b][ds(ni - 1, 2), :, :, :].transpose([1, 0, 2, 3]),
                        in_=stage4[:, :, :, :])

    nc.compile()
    return nc


def _host_prep(inputs):
    import ml_dtypes
    bf = ml_dtypes.bfloat16
    e4 = ml_dtypes.float8_e4m3
    x = np.asarray(inputs["x"], dtype=np.float32)
    edge_index = np.asarray(inputs["edge_index"])
    g_w = np.asarray(inputs["g_norm_w"], dtype=np.float32)
    g_b = np.asarray(inputs["g_norm_b"], dtype=np.float32)
    t_w = np.asarray(inputs["t_norm_w"], dtype=np.float32)
    t_b = np.asarray(inputs["t_norm_b"], dtype=np.float32)
    conv_w = np.asarray(inputs["conv_w"], dtype=np.float32)
    conv_b = np.asarray(inputs["conv_b"], dtype=np.float32)

    # fast path requires LN affine params constant (true for this problem family)
    assert np.all(g_w == g_w.flat[0]) and np.all(t_w == t_w.flat[0]), \
        "non-constant LayerNorm weight not supported by this kernel"
    kg = float(g_w.flat[0])
    kt = float(t_w.flat[0])
    assert np.all(t_b == t_b.flat[0]), "non-constant t_norm_b not supported"
    kb = float(t_b.flat[0])

    src = edge_index[0].astype(np.int64)
    dst = edge_index[1].astype(np.int64)
    deg = np.zeros(N, np.float32)
    np.add.at(deg, dst, np.float32(1.0))
    dinv = np.where(deg > 0, 1.0 / np.sqrt(np.maximum(deg, 1.0)), 0.0).astype(np.float32)

    # keep only edges with nonzero weight (dinv[src] > 0; dst always has deg>=1)
    keep = dinv[src] > 0
    srck, dstk = src[keep], dst[keep]

    # S: integer edge counts, exact in fp8. Row = src, col = dst.
    Sf = np.zeros((N, N), np.float32)
    np.add.at(Sf, (srck, dstk), np.float32(1.0))
    s8 = np.ascontiguousarray(Sf.reshape(128, KP, 2, N)).astype(e4)

    # u1[dst] = sum_e dinv[src_e]; corr folded as (c*G + ncu*u1 [+v1]) * dd
    u1 = np.zeros(N, np.float32)
    np.add.at(u1, dstk, dinv[srck])
    ddr = (kg * dinv).astype(np.float32)
    u1t2 = np.ascontiguousarray(np.broadcast_to(u1, (128, N))).astype(np.float32)
    ddt2 = np.ascontiguousarray(np.broadcast_to(ddr, (128, N))).astype(np.float32)

    # v = A @ g_b (element-wise over h); v1 = v / dd  (guard dd == 0)
    has_v = bool(np.any(g_b != 0))
    if has_v:
        A = np.zeros((N, N), np.float32)
        A[dstk, srck] = 0.0
        np.add.at(A, (dstk, srck), (dinv[srck] * dinv[dstk]).astype(np.float32))
        v = A @ g_b          # [N, H]
        vt2 = np.empty((128, N), np.float32)
        vt2[:64] = v.T; vt2[64:] = v.T
        with np.errstate(divide="ignore", invalid="ignore"):
            v1t2 = np.where(ddt2 != 0, vt2 / ddt2, 0.0).astype(np.float32)
    else:
        v1t2 = np.zeros((1, 1), np.float32)  # unused

    # x scaled by dinv[src], split hi+lo fp8; plus raw bf16 for LN1 stats
    xb = np.ascontiguousarray(
        x.astype(bf).reshape(B, P, 128, KT, H).transpose(0, 2, 3, 1, 4))
    xs = x * dinv[None, None, :, None]
    xs = np.ascontiguousarray(
        xs.reshape(B, P, 128, KP, 2, H).transpose(0, 2, 3, 4, 1, 5))
    x8h = xs.astype(e4)
    x8l = (xs - x8h.astype(np.float32)).astype(e4)

    cwi = np.zeros((PH, PH), np.float32)
    for p in range(P):
        for q in range(P):
            w = conv_w[q, p] * kt
            idx = np.arange(H)
            cwi[p * H + idx, q * H + idx] = w
    cwi = cwi.astype(bf)

    r12 = np.zeros((P, PH), np.float32)
    for p in range(P):
        r12[p, p * H:(p + 1) * H] = 1.0
    bo = np.zeros((PH, P), np.float32)
    for p in range(P):
        bo[p * H:(p + 1) * H, p] = 1.0
    cwt = np.ascontiguousarray(conv_w.T * kt)
    cb = (conv_b + kb * conv_w.sum(axis=1)).astype(np.float32).reshape(P, 1)

    consts = {"s8": s8, "u1": u1t2, "dd": ddt2, "cwi": cwi,
              "r12": r12, "bo": bo, "cwt": cwt, "cb": cb}
    if has_v:
        consts["v1"] = v1t2
    return (xb, x8h, x8l), consts, has_v


def _unpack_out(arr):
    """[BL, KT(ni), 128, P, H] -> [BL, P, N, H] with n = ni*128 + t."""
    return np.ascontiguousarray(arr.transpose(0, 3, 1, 2, 4).reshape(BL, P, N, H))


def kernel(**inputs):
    from concourse.bass_utils import run_bass_kernel_spmd

    (xb, x8h, x8l), consts, has_v = _host_prep(inputs)

    if ("nc", has_v) not in _CACHE:
        _CACHE[("nc", has_v)] = _build_program(has_v)
    nc = _CACHE[("nc", has_v)]

    in_maps = []
    for c in range(NCORES):
        sl = slice(c * BL, (c + 1) * BL)
        m = {"xb": np.ascontiguousarray(xb[sl]),
             "x8h": np.ascontiguousarray(x8h[sl]),
             "x8l": np.ascontiguousarray(x8l[sl])}
        m.update(consts)
        in_maps.append(m)

    res = run_bass_kernel_spmd(nc, in_maps, core_ids=list(range(NCORES)))
    out = np.empty((B, P, N, H), np.float32)
    for c in range(NCORES):
        out[c * BL:(c + 1) * BL] = _unpack_out(res.results[c]["out"])
    return out


# revision 18
# speedup vs baseline: 1.2347x; 1.0566x over previous
"""Trainium2 Bass kernel for nn_CondBlock (LayerNorm -> LightGCN conv -> LayerNorm -> 1x1 conv over P).

Self-contained: hardcoded shapes, host-side graph preprocessing, 8-core
data-parallel (over batch) SPMD execution via run_bass_kernel_spmd.

Algorithm (validated vs reference in fp32):
  per slice s=(b,p): LN1: h1 = c_s*(x - mu_s)*g_w + g_b, c_s = rsqrt(var_s+eps)
  conv:  A = D_dst S D_src with S integer edge counts (exact in fp8).
         Device matmul computes G = S^T @ (dinv_src * x) using fp8e4
         DoubleRow matmuls (hi+lo residual split of the scaled x, both
         accumulated in PSUM -> ~1e-3 precision at 2x bf16 PE rate).
         Evict: Z = (c*G + ncu*u1 [+ v1]) * dd, dd = kg*dinv_dst (per col).
  LN2 + P-mix folded:
         out_q = sum_p aa[q,p]*Z_p + r1[q],  aa[q,p] = conv_w[q,p]*c2_p*kt
         r1[q] = -sum_p aa[q,p]*mu2_p + kb*sum_p conv_w[q,p] + conv_b[q]
  Pass-2 matmul  out[n,(q,h)] = Z^T-tiles^T @ W (bf16), W = (conv_w (x) I_64)*c2*kt
"""

import numpy as np

B, P, N, H = 16, 12, 2048, 64
E = 16384
NCORES = 8
BL = B // NCORES      # batches per core
PH = P * H            # 768
MC = PH // 128        # 6 (p,h)-chunks of 128
KT = N // 128         # 16 node tiles
KP = 8                # DoubleRow src chunk pairs (256 nodes each)
FQW = 512             # dst-column chunk width for pass-1
FQ = N // FQW         # 4
NH = float(N * H)
EPS = 1e-5

_CACHE = {}


def _build_program(has_v=False):
    import os
    SKIP = set(filter(None, os.environ.get("K_SKIP", "").split(",")))
    from concourse import bass, bacc, tile, mybir
    from contextlib import ExitStack

    f32 = mybir.dt.float32
    bf16 = mybir.dt.bfloat16
    fp8 = mybir.dt.float8e4
    ds = bass.ds
    Alu = mybir.AluOpType
    Act = mybir.ActivationFunctionType
    DR = mybir.MatmulPerfMode.DoubleRow

    nc = bacc.Bacc("TRN2", target_bir_lowering=False, debug=False)

    xb_d = nc.dram_tensor("xb", [BL, 128, KT, P, H], bf16, kind="ExternalInput").ap()
    x8h_d = nc.dram_tensor("x8h", [BL, 128, KP, 2, P, H], fp8, kind="ExternalInput").ap()
    x8l_d = nc.dram_tensor("x8l", [BL, 128, KP, 2, P, H], fp8, kind="ExternalInput").ap()
    s8_d = nc.dram_tensor("s8", [128, KP, 2, N], fp8, kind="ExternalInput").ap()
    u1_d = nc.dram_tensor("u1", [128, N], f32, kind="ExternalInput").ap()
    dd_d = nc.dram_tensor("dd", [128, N], f32, kind="ExternalInput").ap()
    v1_d = nc.dram_tensor("v1", [128, N], f32, kind="ExternalInput").ap() if has_v else None
    cwi_d = nc.dram_tensor("cwi", [PH, PH], bf16, kind="ExternalInput").ap()
    r12_d = nc.dram_tensor("r12", [P, PH], f32, kind="ExternalInput").ap()
    bo_d = nc.dram_tensor("bo", [PH, P], f32, kind="ExternalInput").ap()
    cwt_d = nc.dram_tensor("cwt", [P, P], f32, kind="ExternalInput").ap()
    cb_d = nc.dram_tensor("cb", [P, 1], f32, kind="ExternalInput").ap()
    out_d = nc.dram_tensor("out", [BL, KT, 128, P, H], bf16, kind="ExternalOutput").ap()

    with tile.TileContext(nc) as tc, ExitStack() as ctx:
        cons = ctx.enter_context(tc.tile_pool(name="cons", bufs=1))
        xpool = ctx.enter_context(tc.tile_pool(name="xp", bufs=1))
        zpool = ctx.enter_context(tc.tile_pool(name="zp", bufs=1))
        wpool = ctx.enter_context(tc.tile_pool(name="wp", bufs=1))
        sp = ctx.enter_context(tc.tile_pool(name="sp", bufs=2))
        sml = ctx.enter_context(tc.tile_pool(name="sml", bufs=1))
        pp = ctx.enter_context(tc.tile_pool(name="pp", bufs=6, space="PSUM"))

        # ---- constants ----
        u1t = cons.tile([128, N], f32, tag="u1t")
        ddt = cons.tile([128, N], f32, tag="ddt")
        v1t = cons.tile([128, N], f32, tag="v1t") if has_v else None
        r12 = cons.tile([P, PH], f32, tag="r12")
        bo = cons.tile([128, MC, P], f32, tag="bo")
        cwt = cons.tile([P, P], f32, tag="cwt")
        cb = cons.tile([P, 1], f32, tag="cb")
        onesk = cons.tile([128, 1], bf16, tag="onesk")
        onesm = cons.tile([1, 128], f32, tag="onesm")
        def load_consts():
            nc.scalar.dma_start(out=u1t[:, :], in_=u1_d[:, :])
            nc.scalar.dma_start(out=ddt[:, :], in_=dd_d[:, :])
            if has_v:
                nc.scalar.dma_start(out=v1t[:, :], in_=v1_d[:, :])
            nc.scalar.dma_start(out=r12[:, :], in_=r12_d[:, :])
            nc.scalar.dma_start(out=bo[:, :, :], in_=bo_d.rearrange("(c t) p -> t c p", t=128))
            nc.scalar.dma_start(out=cwt[:, :], in_=cwt_d[:, :])
            nc.scalar.dma_start(out=cb[:, :], in_=cb_d[:, :])
        onesf = cons.tile([128, 1], f32, tag="onesf")
        nc.vector.memset(onesf[:, :], 1.0)
        nc.vector.tensor_copy(onesk[:, :], onesf[:, :])
        nc.vector.memset(onesm[:, :], 1.0)

        s8 = ctx.enter_context(tc.tile_pool(name="s8p", bufs=1)).tile(
            [128, KP, 2, N], fp8, tag="S8")

        def load_s8(kcs):
            for kc in kcs:
                nc.sync.dma_start(
                    out=s8[:, ds(2 * kc, 2), :, :],
                    in_=s8_d[:, ds(2 * kc, 2), :, :])

        def mm(out, lhsT, rhs, start, stop):
            nc.tensor.matmul(out, lhsT, rhs, start=start, stop=stop)

        def col12(row):
            """[1,12] sbuf row -> [12,1] sbuf col (via PE)."""
            ps = pp.tile([12, 1], f32, tag="ps")
            mm(ps[:, :], row, onesm[:, 0:1], True, True)
            col = sml.tile([12, 1], f32, tag=None)
            nc.vector.tensor_copy(col[:, :], ps[:, :])
            return col

        def expand12(col_sb, dst):
            """[12,1] sbuf col -> dst [128, MC] per-partition cols (c[p] replicated over h)."""
            for m in range(MC):
                ps = pp.tile([128, 1], f32, tag="ps")
                mm(ps[:, :], r12[:, ds(m * 128, 128)], col_sb, True, True)
                nc.vector.tensor_copy(dst[:, m:m + 1], ps[:, :])

        for b in range(BL):
            # ---- load x: bf16 node-major for stats + fp8 hi/lo for conv ----
            # Order matters: all DMAs serialize on the shared DMA_ENGINES
            # resource, so front-load what gates the earliest compute.
            if b == 0:
                load_s8([0])
            X8h = xpool.tile([128, KP, 2, P, H], fp8, tag="X8h")
            X8l = xpool.tile([128, KP, 2, P, H], fp8, tag="X8l")
            nc.gpsimd.dma_start(out=X8h[:, ds(0, 4), :, :, :],
                                in_=x8h_d[b][:, ds(0, 4), :, :, :])
            nc.gpsimd.dma_start(out=X8l[:, ds(0, 4), :, :, :],
                                in_=x8l_d[b][:, ds(0, 4), :, :, :])
            X = xpool.tile([128, KT, P, H], bf16, tag="X")
            for kh in range(4):
                nc.sync.dma_start(
                    out=X[:, ds(4 * kh, 4), :, :],
                    in_=xb_d[b][:, ds(4 * kh, 4), :, :])
            nc.gpsimd.dma_start(out=X8h[:, ds(4, 4), :, :, :],
                                in_=x8h_d[b][:, ds(4, 4), :, :, :])
            nc.gpsimd.dma_start(out=X8l[:, ds(4, 4), :, :, :],
                                in_=x8l_d[b][:, ds(4, 4), :, :, :])
            if b == 0:
                load_s8([1, 2, 3])
                load_consts()

            # ---- LN1 stats: PE ones-matmuls, x then x^2 (2 psum banks at a time) ----
            NKS = KT if "stats" not in SKIP else 1
            ps_s1 = pp.tile([1, 2, 512], f32, tag="ps2", name=f"ps_s1_{b}", bufs=1)
            for k in range(NKS):
                for hx in range(2):
                    mm(ps_s1[:, hx, 0:384], onesk[:, :],
                       X[:, k, 6 * hx:6 * hx + 6, :], k == 0, k == NKS - 1)
            s1row = sml.tile([1, PH], f32, tag="s1row")
            for hx in range(2):
                nc.vector.tensor_copy(s1row[:, ds(384 * hx, 384)], ps_s1[:, hx, 0:384])
            ps_q1 = pp.tile([1, 2, 512], f32, tag="ps2", name=f"ps_q1_{b}", bufs=1)
            for k in range(NKS):
                sqx = sp.tile([128, P, H], bf16, tag="sqx")
                nc.scalar.activation(sqx[:, :, :], X[:, k, :, :], Act.Square)
                for hx in range(2):
                    mm(ps_q1[:, hx, 0:384], onesk[:, :],
                       sqx[:, 6 * hx:6 * hx + 6, :], k == 0, k == NKS - 1)
            q1row = sml.tile([1, PH], f32, tag="q1row")
            for hx in range(2):
                nc.vector.tensor_copy(q1row[:, ds(384 * hx, 384)], ps_q1[:, hx, 0:384])
            s1p = sml.tile([1, P], f32, tag="s1p")
            q1p = sml.tile([1, P], f32, tag="q1p")
            with nc.allow_low_precision(reason="12-col reduce in f32"):
                nc.vector.tensor_reduce(s1p[:, :], s1row.rearrange("o (p h) -> o p h", h=H),
                                        mybir.AxisListType.X, Alu.add)
                nc.vector.tensor_reduce(q1p[:, :], q1row.rearrange("o (p h) -> o p h", h=H),
                                        mybir.AxisListType.X, Alu.add)
            s1c = col12(s1p[:, :])
            q1c = col12(q1p[:, :])
            # mu, var, c = rsqrt(var+eps), ncu = -c*mu   (all [12,1])
            mu = sml.tile([P, 1], f32, tag="mu")
            var = sml.tile([P, 1], f32, tag="var")
            tmp = sml.tile([P, 1], f32, tag="tmp")
            c12t = sml.tile([P, 1], f32, tag="c12t")
            ncu12 = sml.tile([P, 1], f32, tag="ncu12")
            nc.vector.tensor_scalar(mu[:, :], s1c[:, :], 1.0 / NH, None, Alu.mult)
            nc.vector.tensor_tensor(tmp[:, :], mu[:, :], mu[:, :], Alu.mult)
            nc.vector.tensor_scalar(var[:, :], q1c[:, :], 1.0 / NH, None, Alu.mult)
            nc.vector.tensor_tensor(var[:, :], var[:, :], tmp[:, :], Alu.subtract)
            nc.vector.tensor_scalar(var[:, :], var[:, :], EPS, None, Alu.add)
            nc.vector.reciprocal(tmp[:, :], var[:, :])
            nc.scalar.activation(c12t[:, :], tmp[:, :], Act.Sqrt)
            nc.vector.scalar_tensor_tensor(ncu12[:, :], c12t[:, :], -1.0, mu[:, :],
                                           Alu.mult, Alu.mult)
            c_col = sml.tile([128, MC], f32, tag="c_col")
            ncu_col = sml.tile([128, MC], f32, tag="ncu_col")
            expand12(c12t[:, :], c_col)
            expand12(ncu12[:, :], ncu_col)

            # ---- W staging: DMA CWI now (scaled by c2 later) ----
            W = wpool.tile([128, MC, PH], bf16, tag="W")
            nc.scalar.dma_start(out=W[:, :, :], in_=cwi_d.rearrange("(c t) f -> t c f", t=128))

            # ---- pass-1 conv: G = S^T @ (dinv_src*x) via fp8 DoubleRow hi+lo ----
            Z = zpool.tile([128, MC, N], bf16, tag="Z")
            zs_slots = sml.tile([128, MC, FQ], f32, tag="zs")
            zq_slots = sml.tile([128, MC, FQ], f32, tag="zq")
            for fq in range(FQ):
                gps = [pp.tile([128, FQW], f32, tag="ps", name=f"gps_{b}_{fq}_{i}") for i in range(MC)]
                NKC = KP if "conv" not in SKIP else 1
                if fq == 0:
                    # kp-outer: consume s8/x8 chunks as their DMAs land
                    for kp in range(NKC):
                        for si, Xs in enumerate((X8h, X8l)):
                            for m in range(MC):
                                nc.tensor.matmul(
                                    gps[m][:, :], Xs[:, kp, :, 2 * m:2 * m + 2, :],
                                    s8[:, kp, :, ds(fq * FQW, FQW)],
                                    start=(kp == 0 and si == 0),
                                    stop=(kp == NKC - 1 and si == 1),
                                    perf_mode=DR)
                else:
                    # m-outer: finish chunk m early so its evict pipelines
                    # under the remaining matmuls
                    for m in range(MC):
                        for kp in range(NKC):
                            for si, Xs in enumerate((X8h, X8l)):
                                nc.tensor.matmul(
                                    gps[m][:, :], Xs[:, kp, :, 2 * m:2 * m + 2, :],
                                    s8[:, kp, :, ds(fq * FQW, FQW)],
                                    start=(kp == 0 and si == 0),
                                    stop=(kp == NKC - 1 and si == 1),
                                    perf_mode=DR)
                for m in range(MC if "evict" not in SKIP else 0):
                    corr = sp.tile([128, FQW], f32, tag="corr")
                    if has_v:
                        nc.vector.scalar_tensor_tensor(
                            corr[:, :], u1t[:, ds(fq * FQW, FQW)], ncu_col[:, m:m + 1],
                            v1t[:, ds(fq * FQW, FQW)], Alu.mult, Alu.add)
                    else:
                        nc.scalar.activation(corr[:, :], u1t[:, ds(fq * FQW, FQW)],
                                             Act.Copy, scale=ncu_col[:, m:m + 1])
                    zt = sp.tile([128, FQW], f32, tag="zt")
                    nc.vector.scalar_tensor_tensor(
                        zt[:, :], gps[m][:, :], c_col[:, m:m + 1],
                        corr[:, :], Alu.mult, Alu.add)
                    with nc.allow_low_precision(reason="Z stored bf16 for pass-2"):
                        nc.vector.scalar_tensor_tensor(
                            Z[:, m, ds(fq * FQW, FQW)], zt[:, :], 1.0,
                            ddt[:, ds(fq * FQW, FQW)], Alu.mult, Alu.mult,
                            accum_out=zs_slots[:, m, fq:fq + 1])
                    sqz = sp.tile([128, FQW], f32, tag="sqz")
                    zsl = Z[:, m, ds(fq * FQW, FQW)]
                    nc.scalar.activation(sqz[:, :], zsl, Act.Square,
                                         accum_out=zq_slots[:, m, fq:fq + 1])

            # ---- LN2 stats ----
            zs6 = sml.tile([128, MC], f32, tag="zs6")
            zq6 = sml.tile([128, MC], f32, tag="zq6")
            with nc.allow_low_precision(reason="4-col reduce in f32"):
                nc.vector.tensor_reduce(zs6[:, :], zs_slots[:, :, :], mybir.AxisListType.X, Alu.add)
                nc.vector.tensor_reduce(zq6[:, :], zq_slots[:, :, :], mybir.AxisListType.X, Alu.add)
            ps_s2 = pp.tile([P, 1], f32, tag="ps")
            ps_q2 = pp.tile([P, 1], f32, tag="ps")
            for m in range(MC):
                mm(ps_s2[:, :], bo[:, m, :], zs6[:, m:m + 1], m == 0, m == MC - 1)
                mm(ps_q2[:, :], bo[:, m, :], zq6[:, m:m + 1], m == 0, m == MC - 1)
            s2c = sml.tile([P, 1], f32, tag="s2c")
            q2c = sml.tile([P, 1], f32, tag="q2c")
            nc.vector.tensor_copy(s2c[:, :], ps_s2[:, :])
            nc.vector.tensor_copy(q2c[:, :], ps_q2[:, :])
            mu2 = sml.tile([P, 1], f32, tag="mu2")
            var2 = sml.tile([P, 1], f32, tag="var2")
            tmp2 = sml.tile([P, 1], f32, tag="tmp2")
            c2t = sml.tile([P, 1], f32, tag="c2t")
            nc.vector.tensor_scalar(mu2[:, :], s2c[:, :], 1.0 / NH, None, Alu.mult)
            nc.vector.tensor_tensor(tmp2[:, :], mu2[:, :], mu2[:, :], Alu.mult)
            nc.vector.tensor_scalar(var2[:, :], q2c[:, :], 1.0 / NH, None, Alu.mult)
            nc.vector.tensor_tensor(var2[:, :], var2[:, :], tmp2[:, :], Alu.subtract)
            nc.vector.tensor_scalar(var2[:, :], var2[:, :], EPS, None, Alu.add)
            nc.vector.reciprocal(tmp2[:, :], var2[:, :])
            nc.scalar.activation(c2t[:, :], tmp2[:, :], Act.Sqrt)
            c2_col = sml.tile([128, MC], f32, tag="c2col")
            expand12(c2t[:, :], c2_col)
            # W = CWI * c2 (per-partition scale)
            with nc.allow_low_precision(reason="W bf16"):
                for m in range(MC):
                    nc.gpsimd.tensor_scalar(W[:, m, :], W[:, m, :], c2_col[:, m:m + 1],
                                            None, Alu.mult)

            def emit_r1():
                # r1[q] = cb[q] - sum_p A1[p,q]*mu2[p],  A1 = cwt*c2
                a1 = sml.tile([P, P], f32, tag="a1")
                nc.vector.tensor_scalar(a1[:, :], cwt[:, :], c2t[:, :], None, Alu.mult)
                ps_k1 = pp.tile([P, 1], f32, tag="ps2", bufs=1, name="ps_k1_r1")
                mm(ps_k1[:, :], a1[:, :], mu2[:, :], True, True)
                r1c = sml.tile([P, 1], f32, tag="r1c")
                nc.vector.tensor_tensor(r1c[:, :], cb[:, :], ps_k1[:, :], Alu.subtract)
                r1row = sml.tile([1, PH], f32, tag="r1row")
                r1B = sml.tile([128, PH], f32, tag="r1B")
                for hx in range(2):
                    psr = pp.tile([1, 384], f32, tag="ps2", bufs=1, name=f"psr_{hx}")
                    mm(psr[:, :], r1c[:, :], r12[:, ds(384 * hx, 384)], True, True)
                    nc.vector.tensor_copy(r1row[:, ds(384 * hx, 384)], psr[:, :])
                for hx in range(2):
                    psb = pp.tile([128, 384], f32, tag="ps2", bufs=1, name=f"psb_{hx}")
                    mm(psb[:, :], onesm[:, :], r1row[:, ds(384 * hx, 384)], True, True)
                    nc.vector.tensor_copy(r1B[:, ds(384 * hx, 384)], psb[:, :])
                return r1B

            # ---- pass-2: out[n, (q,h)] = sum_c Z[:, c, n]^T @ W[:, c, :] ----
            r1B = None
            for ni in range(KT):
                po = [pp.tile([128, 384], f32, tag="ps", name=f"po_{b}_{ni}_{i}") for i in range(2)]
                for kc in range(MC if "pass2" not in SKIP else 1):
                    for hx in range(2):
                        mm(po[hx][:, :], Z[:, kc, ds(ni * 128, 128)],
                           W[:, kc, ds(384 * hx, 384)], kc == 0,
                           (kc == MC - 1 or "pass2" in SKIP))
                if r1B is None:
                    r1B = emit_r1()
                if ni % 2 == 0:
                    stage4 = sp.tile([128, 2, P, H], bf16, tag="ostage")
                with nc.allow_low_precision(reason="out stored bf16"):
                    for hx in range(2):
                        nc.vector.tensor_tensor(
                            stage4[:, ni % 2, ds(6 * hx, 6), :],
                            po[hx].rearrange("t (p h) -> t p h", h=H),
                            r1B[:, ds(384 * hx, 384)].rearrange("t (p h) -> t p h", h=H),
                            Alu.add)
                if "out" not in SKIP and ni >= KT - 2:
                    eng = nc.scalar if ni % 2 == 0 else nc.gpsimd
                    eng.dma_start(
                        out=out_d[b][ni, :, :, :],
                        in_=stage4[:, ni % 2, :, :])
                elif "out" not in SKIP and ni % 2 == 1:
                    eng = nc.scalar if (ni // 2) % 2 == 0 else nc.gpsimd
                    eng.dma_start(
                        out=out_d[b][ds(ni - 1, 2), :, :, :].transpose([1, 0, 2, 3]),
                        in_=stage4[:, :, :, :])

    nc.compile()
    return nc


def _host_prep(inputs):
    import ml_dtypes
    bf = ml_dtypes.bfloat16
    e4 = ml_dtypes.float8_e4m3
    x = np.asarray(inputs["x"], dtype=np.float32)
    edge_index = np.asarray(inputs["edge_index"])
    g_w = np.asarray(inputs["g_norm_w"], dtype=np.float32)
    g_b = np.asarray(inputs["g_norm_b"], dtype=np.float32)
    t_w = np.asarray(inputs["t_norm_w"], dtype=np.float32)
    t_b = np.asarray(inputs["t_norm_b"], dtype=np.float32)
    conv_w = np.asarray(inputs["conv_w"], dtype=np.float32)
    conv_b = np.asarray(inputs["conv_b"], dtype=np.float32)

    # fast path requires LN affine params constant (true for this problem family)
    assert np.all(g_w == g_w.flat[0]) and np.all(t_w == t_w.flat[0]), \
        "non-constant LayerNorm weight not supported by this kernel"
    kg = float(g_w.flat[0])
    kt = float(t_w.flat[0])
    assert np.all(t_b == t_b.flat[0]), "non-constant t_norm_b not supported"
    kb = float(t_b.flat[0])

    src = edge_index[0].astype(np.int64)
    dst = edge_index[1].astype(np.int64)
    deg = np.zeros(N, np.float32)
    np.add.at(deg, dst, np.float32(1.0))
    dinv = np.where(deg > 0, 1.0 / np.sqrt(np.maximum(deg, 1.0)), 0.0).astype(np.float32)

    # keep only edges with nonzero weight (dinv[src] > 0; dst always has deg>=1)
    keep = dinv[src] > 0
    srck, dstk = src[keep], dst[keep]

    # S: integer edge counts, exact in fp8. Row = src, col = dst.
    Sf = np.zeros((N, N), np.float32)
    np.add.at(Sf, (srck, dstk), np.float32(1.0))
    s8 = np.ascontiguousarray(Sf.reshape(128, KP, 2, N)).astype(e4)

    # u1[dst] = sum_e dinv[src_e]; corr folded as (c*G + ncu*u1 [+v1]) * dd
    u1 = np.zeros(N, np.float32)
    np.add.at(u1, dstk, dinv[srck])
    ddr = (kg * dinv).astype(np.float32)
    u1t2 = np.ascontiguousarray(np.broadcast_to(u1, (128, N))).astype(np.float32)
    ddt2 = np.ascontiguousarray(np.broadcast_to(ddr, (128, N))).astype(np.float32)

    # v = A @ g_b (element-wise over h); v1 = v / dd  (guard dd == 0)
    has_v = bool(np.any(g_b != 0))
    if has_v:
        A = np.zeros((N, N), np.float32)
        A[dstk, srck] = 0.0
        np.add.at(A, (dstk, srck), (dinv[srck] * dinv[dstk]).astype(np.float32))
        v = A @ g_b          # [N, H]
        vt2 = np.empty((128, N), np.float32)
        vt2[:64] = v.T; vt2[64:] = v.T
        with np.errstate(divide="ignore", invalid="ignore"):
            v1t2 = np.where(ddt2 != 0, vt2 / ddt2, 0.0).astype(np.float32)
    else:
        v1t2 = np.zeros((1, 1), np.float32)  # unused

    # x scaled by dinv[src], split hi+lo fp8; plus raw bf16 for LN1 stats
    xb = np.ascontiguousarray(
        x.astype(bf).reshape(B, P, 128, KT, H).transpose(0, 2, 3, 1, 4))
    xs = x * dinv[None, None, :, None]
    xs = np.ascontiguousarray(
        xs.reshape(B, P, 128, KP, 2, H).transpose(0, 2, 3, 4, 1, 5))
    x8h = xs.astype(e4)
    x8l = (xs - x8h.astype(np.float32)).astype(e4)

    cwi = np.zeros((PH, PH), np.float32)
    for p in range(P):
        for q in range(P):
            w = conv_w[q, p] * kt
            idx = np.arange(H)
            cwi[p * H + idx, q * H + idx] = w
    cwi = cwi.astype(bf)

    r12 = np.zeros((P, PH), np.float32)
    for p in range(P):
        r12[p, p * H:(p + 1) * H] = 1.0
    bo = np.zeros((PH, P), np.float32)
    for p in range(P):
        bo[p * H:(p + 1) * H, p] = 1.0
    cwt = np.ascontiguousarray(conv_w.T * kt)
    cb = (conv_b + kb * conv_w.sum(axis=1)).astype(np.float32).reshape(P, 1)

    consts = {"s8": s8, "u1": u1t2, "dd": ddt2, "cwi": cwi,
              "r12": r12, "bo": bo, "cwt": cwt, "cb": cb}
    if has_v:
        consts["v1"] = v1t2
    return (xb, x8h, x8l), consts, has_v


def _unpack_out(arr):
    """[BL, KT(ni), 128, P, H] bf16 -> [BL, P, N, H] f32 with n = ni*128 + t."""
    return np.ascontiguousarray(
        arr.astype(np.float32).transpose(0, 3, 1, 2, 4).reshape(BL, P, N, H))


def kernel(**inputs):
    from concourse.bass_utils import run_bass_kernel_spmd

    (xb, x8h, x8l), consts, has_v = _host_prep(inputs)

    if ("nc", has_v) not in _CACHE:
        _CACHE[("nc", has_v)] = _build_program(has_v)
    nc = _CACHE[("nc", has_v)]

    in_maps = []
    for c in range(NCORES):
        sl = slice(c * BL, (c + 1) * BL)
        m = {"xb": np.ascontiguousarray(xb[sl]),
             "x8h": np.ascontiguousarray(x8h[sl]),
             "x8l": np.ascontiguousarray(x8l[sl])}
        m.update(consts)
        in_maps.append(m)

    res = run_bass_kernel_spmd(nc, in_maps, core_ids=list(range(NCORES)))
    out = np.empty((B, P, N, H), np.float32)
    for c in range(NCORES):
        out[c * BL:(c + 1) * BL] = _unpack_out(res.results[c]["out"])
    return out


# revision 21
# speedup vs baseline: 1.2663x; 1.0256x over previous
"""Trainium2 Bass kernel for nn_CondBlock (LayerNorm -> LightGCN conv -> LayerNorm -> 1x1 conv over P).

Self-contained: hardcoded shapes, host-side graph preprocessing, 8-core
data-parallel (over batch) SPMD execution via run_bass_kernel_spmd.

Algorithm (validated vs reference in fp32):
  per slice s=(b,p): LN1: h1 = c_s*(x - mu_s)*g_w + g_b, c_s = rsqrt(var_s+eps)
  conv:  A = D_dst S D_src with S integer edge counts (exact in fp8).
         Device matmul computes G = S^T @ (dinv_src * x) using fp8e4
         DoubleRow matmuls (hi+lo residual split of the scaled x, both
         accumulated in PSUM -> ~1e-3 precision at 2x bf16 PE rate).
         Evict: Z = (c*G + ncu*u1 [+ v1]) * dd, dd = kg*dinv_dst (per col).
  LN2 + P-mix folded:
         out_q = sum_p aa[q,p]*Z_p + r1[q],  aa[q,p] = conv_w[q,p]*c2_p*kt
         r1[q] = -sum_p aa[q,p]*mu2_p + kb*sum_p conv_w[q,p] + conv_b[q]
  Pass-2 matmul  out[n,(q,h)] = Z^T-tiles^T @ W (bf16), W = (conv_w (x) I_64)*c2*kt
"""

import numpy as np

B, P, N, H = 16, 12, 2048, 64
E = 16384
NCORES = 8
BL = B // NCORES      # batches per core
PH = P * H            # 768
MC = PH // 128        # 6 (p,h)-chunks of 128
KT = N // 128         # 16 node tiles
KP = 8                # DoubleRow src chunk pairs (256 nodes each)
FQW = 512             # dst-column chunk width for pass-1
FQ = N // FQW         # 4
NH = float(N * H)
EPS = 1e-5

_CACHE = {}


def _build_program(has_v=False):
    import os
    SKIP = set(filter(None, os.environ.get("K_SKIP", "").split(",")))
    from concourse import bass, bacc, tile, mybir
    from contextlib import ExitStack

    f32 = mybir.dt.float32
    bf16 = mybir.dt.bfloat16
    fp8 = mybir.dt.float8e4
    ds = bass.ds
    Alu = mybir.AluOpType
    Act = mybir.ActivationFunctionType
    DR = mybir.MatmulPerfMode.DoubleRow

    nc = bacc.Bacc("TRN2", target_bir_lowering=False, debug=False)

    xb_d = nc.dram_tensor("xb", [BL, 128, KT, P, H], bf16, kind="ExternalInput").ap()
    x8h_d = nc.dram_tensor("x8h", [BL, 128, KP, 2, P, H], fp8, kind="ExternalInput").ap()
    x8l_d = nc.dram_tensor("x8l", [BL, 128, KP, 2, P, H], fp8, kind="ExternalInput").ap()
    s8_d = nc.dram_tensor("s8", [128, KP, 2, N], fp8, kind="ExternalInput").ap()
    u1_d = nc.dram_tensor("u1", [128, N], f32, kind="ExternalInput").ap()
    dd_d = nc.dram_tensor("dd", [128, N], f32, kind="ExternalInput").ap()
    v1_d = nc.dram_tensor("v1", [128, N], f32, kind="ExternalInput").ap() if has_v else None
    cwi_d = nc.dram_tensor("cwi", [PH, PH], bf16, kind="ExternalInput").ap()
    r12_d = nc.dram_tensor("r12", [P, PH], f32, kind="ExternalInput").ap()
    bo_d = nc.dram_tensor("bo", [PH, P], f32, kind="ExternalInput").ap()
    cwt_d = nc.dram_tensor("cwt", [P, P], f32, kind="ExternalInput").ap()
    cb_d = nc.dram_tensor("cb", [P, 1], f32, kind="ExternalInput").ap()
    out_d = nc.dram_tensor("out", [BL, KT, 128, P, H], bf16, kind="ExternalOutput").ap()

    with tile.TileContext(nc) as tc, ExitStack() as ctx:
        cons = ctx.enter_context(tc.tile_pool(name="cons", bufs=1))
        xpool = ctx.enter_context(tc.tile_pool(name="xp", bufs=1))
        zpool = ctx.enter_context(tc.tile_pool(name="zp", bufs=1))
        wpool = ctx.enter_context(tc.tile_pool(name="wp", bufs=1))
        sp = ctx.enter_context(tc.tile_pool(name="sp", bufs=2))
        sml = ctx.enter_context(tc.tile_pool(name="sml", bufs=1))
        pp = ctx.enter_context(tc.tile_pool(name="pp", bufs=6, space="PSUM"))

        # ---- constants ----
        u1t = cons.tile([128, N], f32, tag="u1t")
        ddt = cons.tile([128, N], f32, tag="ddt")
        v1t = cons.tile([128, N], f32, tag="v1t") if has_v else None
        r12 = cons.tile([P, PH], f32, tag="r12")
        bo = cons.tile([128, MC, P], f32, tag="bo")
        cwt = cons.tile([P, P], f32, tag="cwt")
        cb = cons.tile([P, 1], f32, tag="cb")
        onesk = cons.tile([128, 1], bf16, tag="onesk")
        onesm = cons.tile([1, 128], f32, tag="onesm")
        def load_consts():
            nc.scalar.dma_start(out=u1t[:, :], in_=u1_d[:, :])
            nc.scalar.dma_start(out=ddt[:, :], in_=dd_d[:, :])
            if has_v:
                nc.scalar.dma_start(out=v1t[:, :], in_=v1_d[:, :])
            nc.scalar.dma_start(out=r12[:, :], in_=r12_d[:, :])
            nc.scalar.dma_start(out=bo[:, :, :], in_=bo_d.rearrange("(c t) p -> t c p", t=128))
            nc.scalar.dma_start(out=cwt[:, :], in_=cwt_d[:, :])
            nc.scalar.dma_start(out=cb[:, :], in_=cb_d[:, :])
        onesf = cons.tile([128, 1], f32, tag="onesf")
        nc.vector.memset(onesf[:, :], 1.0)
        nc.vector.tensor_copy(onesk[:, :], onesf[:, :])
        nc.vector.memset(onesm[:, :], 1.0)

        s8 = ctx.enter_context(tc.tile_pool(name="s8p", bufs=1)).tile(
            [128, KP, 2, N], fp8, tag="S8")

        def load_s8(kcs):
            for kc in kcs:
                nc.sync.dma_start(
                    out=s8[:, ds(2 * kc, 2), :, :],
                    in_=s8_d[:, ds(2 * kc, 2), :, :])

        def mm(out, lhsT, rhs, start, stop):
            nc.tensor.matmul(out, lhsT, rhs, start=start, stop=stop)

        def col12(row):
            """[1,12] sbuf row -> [12,1] sbuf col (via PE)."""
            ps = pp.tile([12, 1], f32, tag="ps")
            mm(ps[:, :], row, onesm[:, 0:1], True, True)
            col = sml.tile([12, 1], f32, tag=None)
            nc.vector.tensor_copy(col[:, :], ps[:, :])
            return col

        def expand12(col_sb, dst):
            """[12,1] sbuf col -> dst [128, MC] per-partition cols (c[p] replicated over h)."""
            for m in range(MC):
                ps = pp.tile([128, 1], f32, tag="ps")
                mm(ps[:, :], r12[:, ds(m * 128, 128)], col_sb, True, True)
                nc.vector.tensor_copy(dst[:, m:m + 1], ps[:, :])

        def emit_x8loads(b, X8h, X8l):
            for kh in range(4):
                nc.gpsimd.dma_start(out=X8h[:, ds(2 * kh, 2), :, :, :],
                                    in_=x8h_d[b][:, ds(2 * kh, 2), :, :, :])
                nc.gpsimd.dma_start(out=X8l[:, ds(2 * kh, 2), :, :, :],
                                    in_=x8l_d[b][:, ds(2 * kh, 2), :, :, :])

        def emit_xbload(b, X):
            for kh in range(4):
                nc.sync.dma_start(
                    out=X[:, ds(4 * kh, 4), :, :],
                    in_=xb_d[b][:, ds(4 * kh, 4), :, :])

        def emit_stats(b, X):
            """LN1 stats for batch b -> (c_col, ncu_col) [128, MC] tiles."""
            NKS = KT if "stats" not in SKIP else 1
            ps_s1 = pp.tile([1, 2, 512], f32, tag="ps2", name=f"ps_s1_{b}", bufs=1)
            for k in range(NKS):
                for hx in range(2):
                    mm(ps_s1[:, hx, 0:384], onesk[:, :],
                       X[:, k, 6 * hx:6 * hx + 6, :], k == 0, k == NKS - 1)
            s1row = sml.tile([1, PH], f32, tag="s1row")
            for hx in range(2):
                nc.vector.tensor_copy(s1row[:, ds(384 * hx, 384)], ps_s1[:, hx, 0:384])
            ps_q1 = pp.tile([1, 2, 512], f32, tag="ps2", name=f"ps_q1_{b}", bufs=1)
            for k in range(NKS):
                sqx = sp.tile([128, P, H], bf16, tag="sqx")
                nc.scalar.activation(sqx[:, :, :], X[:, k, :, :], Act.Square)
                for hx in range(2):
                    mm(ps_q1[:, hx, 0:384], onesk[:, :],
                       sqx[:, 6 * hx:6 * hx + 6, :], k == 0, k == NKS - 1)
            q1row = sml.tile([1, PH], f32, tag="q1row")
            for hx in range(2):
                nc.vector.tensor_copy(q1row[:, ds(384 * hx, 384)], ps_q1[:, hx, 0:384])
            s1p = sml.tile([1, P], f32, tag="s1p")
            q1p = sml.tile([1, P], f32, tag="q1p")
            with nc.allow_low_precision(reason="12-col reduce in f32"):
                nc.vector.tensor_reduce(s1p[:, :], s1row.rearrange("o (p h) -> o p h", h=H),
                                        mybir.AxisListType.X, Alu.add)
                nc.vector.tensor_reduce(q1p[:, :], q1row.rearrange("o (p h) -> o p h", h=H),
                                        mybir.AxisListType.X, Alu.add)
            s1c = col12(s1p[:, :])
            q1c = col12(q1p[:, :])
            # mu, var, c = rsqrt(var+eps), ncu = -c*mu   (all [12,1])
            mu = sml.tile([P, 1], f32, tag="mu")
            var = sml.tile([P, 1], f32, tag="var")
            tmp = sml.tile([P, 1], f32, tag="tmp")
            c12t = sml.tile([P, 1], f32, tag="c12t")
            ncu12 = sml.tile([P, 1], f32, tag="ncu12")
            nc.vector.tensor_scalar(mu[:, :], s1c[:, :], 1.0 / NH, None, Alu.mult)
            nc.vector.tensor_tensor(tmp[:, :], mu[:, :], mu[:, :], Alu.mult)
            nc.vector.scalar_tensor_tensor(var[:, :], q1c[:, :], 1.0 / NH, tmp[:, :],
                                           Alu.mult, Alu.subtract)
            nc.vector.tensor_scalar(var[:, :], var[:, :], EPS, None, Alu.add)
            nc.vector.reciprocal(tmp[:, :], var[:, :])
            nc.scalar.activation(c12t[:, :], tmp[:, :], Act.Sqrt)
            nc.vector.scalar_tensor_tensor(ncu12[:, :], c12t[:, :], -1.0, mu[:, :],
                                           Alu.mult, Alu.mult)
            c_col = sml.tile([128, MC], f32, tag="c_col", bufs=2)
            ncu_col = sml.tile([128, MC], f32, tag="ncu_col", bufs=2)
            expand12(c12t[:, :], c_col)
            expand12(ncu12[:, :], ncu_col)
            return c_col, ncu_col

        nxt = None
        for b in range(BL):
            X8h = xpool.tile([128, KP, 2, P, H], fp8, tag="X8h")
            X8l = xpool.tile([128, KP, 2, P, H], fp8, tag="X8l")
            if b == 0:
                X = xpool.tile([128, KT, P, H], bf16, tag="X", bufs=2)
                load_s8([0])
                nc.gpsimd.dma_start(out=X8h[:, ds(0, 2), :, :, :],
                                    in_=x8h_d[0][:, ds(0, 2), :, :, :])
                nc.gpsimd.dma_start(out=X8l[:, ds(0, 2), :, :, :],
                                    in_=x8l_d[0][:, ds(0, 2), :, :, :])
                emit_xbload(0, X)
                load_s8([1, 2, 3])
                for kh in range(1, 4):
                    nc.gpsimd.dma_start(out=X8h[:, ds(2 * kh, 2), :, :, :],
                                        in_=x8h_d[0][:, ds(2 * kh, 2), :, :, :])
                    nc.gpsimd.dma_start(out=X8l[:, ds(2 * kh, 2), :, :, :],
                                        in_=x8l_d[0][:, ds(2 * kh, 2), :, :, :])
                load_consts()
                c_col, ncu_col = emit_stats(0, X)
            else:
                X, c_col, ncu_col = nxt
                emit_x8loads(b, X8h, X8l)

            # ---- W staging: DMA CWI now (scaled by c2 later) ----
            W = wpool.tile([128, MC, PH], bf16, tag="W")
            nc.scalar.dma_start(out=W[:, :, :], in_=cwi_d.rearrange("(c t) f -> t c f", t=128))

            # ---- pass-1 conv: G = S^T @ (dinv_src*x) via fp8 DoubleRow hi+lo ----
            Z = zpool.tile([128, MC, N], bf16, tag="Z")
            zs_slots = sml.tile([128, MC, FQ], f32, tag="zs")
            zq_slots = sml.tile([128, MC, FQ], f32, tag="zq")
            for fq in range(FQ):
                gps = [pp.tile([128, FQW], f32, tag="ps", name=f"gps_{b}_{fq}_{i}") for i in range(MC)]
                NKC = KP if "conv" not in SKIP else 1
                if fq == 0:
                    # kp-outer: consume s8/x8 chunks as their DMAs land
                    for kp in range(NKC):
                        for si, Xs in enumerate((X8h, X8l)):
                            for m in range(MC):
                                nc.tensor.matmul(
                                    gps[m][:, :], Xs[:, kp, :, 2 * m:2 * m + 2, :],
                                    s8[:, kp, :, ds(fq * FQW, FQW)],
                                    start=(kp == 0 and si == 0),
                                    stop=(kp == NKC - 1 and si == 1),
                                    perf_mode=DR)
                else:
                    # m-outer: finish chunk m early so its evict pipelines
                    # under the remaining matmuls
                    for m in range(MC):
                        for kp in range(NKC):
                            for si, Xs in enumerate((X8h, X8l)):
                                nc.tensor.matmul(
                                    gps[m][:, :], Xs[:, kp, :, 2 * m:2 * m + 2, :],
                                    s8[:, kp, :, ds(fq * FQW, FQW)],
                                    start=(kp == 0 and si == 0),
                                    stop=(kp == NKC - 1 and si == 1),
                                    perf_mode=DR)
                for m in range(MC if "evict" not in SKIP else 0):
                    corr = sp.tile([128, FQW], f32, tag="corr")
                    if has_v:
                        nc.vector.scalar_tensor_tensor(
                            corr[:, :], u1t[:, ds(fq * FQW, FQW)], ncu_col[:, m:m + 1],
                            v1t[:, ds(fq * FQW, FQW)], Alu.mult, Alu.add)
                    else:
                        nc.scalar.activation(corr[:, :], u1t[:, ds(fq * FQW, FQW)],
                                             Act.Copy, scale=ncu_col[:, m:m + 1])
                    zt = sp.tile([128, FQW], f32, tag="zt")
                    nc.vector.scalar_tensor_tensor(
                        zt[:, :], gps[m][:, :], c_col[:, m:m + 1],
                        corr[:, :], Alu.mult, Alu.add)
                    with nc.allow_low_precision(reason="Z stored bf16 for pass-2"):
                        nc.vector.scalar_tensor_tensor(
                            Z[:, m, ds(fq * FQW, FQW)], zt[:, :], 1.0,
                            ddt[:, ds(fq * FQW, FQW)], Alu.mult, Alu.mult,
                            accum_out=zs_slots[:, m, fq:fq + 1])
                    sqz = sp.tile([128, FQW], f32, tag="sqz")
                    zsl = Z[:, m, ds(fq * FQW, FQW)]
                    nc.scalar.activation(sqz[:, :], zsl, Act.Square,
                                         accum_out=zq_slots[:, m, fq:fq + 1])

            # ---- LN2 stats ----
            zs6 = sml.tile([128, MC], f32, tag="zs6")
            zq6 = sml.tile([128, MC], f32, tag="zq6")
            with nc.allow_low_precision(reason="4-col reduce in f32"):
                nc.vector.tensor_reduce(zs6[:, :], zs_slots[:, :, :], mybir.AxisListType.X, Alu.add)
                nc.vector.tensor_reduce(zq6[:, :], zq_slots[:, :, :], mybir.AxisListType.X, Alu.add)
            ps_s2 = pp.tile([P, 1], f32, tag="ps")
            ps_q2 = pp.tile([P, 1], f32, tag="ps")
            for m in range(MC):
                mm(ps_s2[:, :], bo[:, m, :], zs6[:, m:m + 1], m == 0, m == MC - 1)
                mm(ps_q2[:, :], bo[:, m, :], zq6[:, m:m + 1], m == 0, m == MC - 1)
            s2c = sml.tile([P, 1], f32, tag="s2c")
            q2c = sml.tile([P, 1], f32, tag="q2c")
            nc.vector.tensor_copy(s2c[:, :], ps_s2[:, :])
            nc.vector.tensor_copy(q2c[:, :], ps_q2[:, :])
            mu2 = sml.tile([P, 1], f32, tag="mu2")
            var2 = sml.tile([P, 1], f32, tag="var2")
            tmp2 = sml.tile([P, 1], f32, tag="tmp2")
            c2t = sml.tile([P, 1], f32, tag="c2t")
            nc.vector.tensor_scalar(mu2[:, :], s2c[:, :], 1.0 / NH, None, Alu.mult)
            nc.vector.tensor_tensor(tmp2[:, :], mu2[:, :], mu2[:, :], Alu.mult)
            nc.vector.scalar_tensor_tensor(var2[:, :], q2c[:, :], 1.0 / NH, tmp2[:, :],
                                           Alu.mult, Alu.subtract)
            nc.vector.tensor_scalar(var2[:, :], var2[:, :], EPS, None, Alu.add)
            nc.vector.reciprocal(tmp2[:, :], var2[:, :])
            nc.scalar.activation(c2t[:, :], tmp2[:, :], Act.Sqrt)
            c2_col = sml.tile([128, MC], f32, tag="c2col")
            expand12(c2t[:, :], c2_col)
            # W = CWI * c2 (per-partition scale)
            with nc.allow_low_precision(reason="W bf16"):
                for m in range(MC):
                    nc.gpsimd.tensor_scalar(W[:, m, :], W[:, m, :], c2_col[:, m:m + 1],
                                            None, Alu.mult)

            def emit_r1():
                # r1[q] = cb[q] - sum_p A1[p,q]*mu2[p],  A1 = cwt*c2
                a1 = sml.tile([P, P], f32, tag="a1")
                nc.vector.tensor_scalar(a1[:, :], cwt[:, :], c2t[:, :], None, Alu.mult)
                ps_k1 = pp.tile([P, 1], f32, tag="ps2", bufs=1, name="ps_k1_r1")
                mm(ps_k1[:, :], a1[:, :], mu2[:, :], True, True)
                r1c = sml.tile([P, 1], f32, tag="r1c")
                nc.vector.tensor_tensor(r1c[:, :], cb[:, :], ps_k1[:, :], Alu.subtract)
                r1row = sml.tile([1, PH], f32, tag="r1row")
                r1B = sml.tile([128, PH], f32, tag="r1B")
                for hx in range(2):
                    psr = pp.tile([1, 384], f32, tag="ps2", bufs=1, name=f"psr_{hx}")
                    mm(psr[:, :], r1c[:, :], r12[:, ds(384 * hx, 384)], True, True)
                    nc.vector.tensor_copy(r1row[:, ds(384 * hx, 384)], psr[:, :])
                for hx in range(2):
                    psb = pp.tile([128, 384], f32, tag="ps2", bufs=1, name=f"psb_{hx}")
                    mm(psb[:, :], onesm[:, :], r1row[:, ds(384 * hx, 384)], True, True)
                    nc.vector.tensor_copy(r1B[:, ds(384 * hx, 384)], psb[:, :])
                return r1B

            # ---- prefetch next batch: X loads + LN1 stats before pass-2 ----
            if b + 1 < BL:
                X_n = xpool.tile([128, KT, P, H], bf16, tag="X", bufs=2)
                emit_xbload(b + 1, X_n)
                cc_n, nc_n = emit_stats(b + 1, X_n)
                nxt = (X_n, cc_n, nc_n)

            # ---- pass-2: out[n, (q,h)] = sum_c Z[:, c, n]^T @ W[:, c, :] ----
            r1B = None
            for ni in range(KT):
                po = [pp.tile([128, 384], f32, tag="ps", name=f"po_{b}_{ni}_{i}") for i in range(2)]
                for kc in range(MC if "pass2" not in SKIP else 1):
                    for hx in range(2):
                        mm(po[hx][:, :], Z[:, kc, ds(ni * 128, 128)],
                           W[:, kc, ds(384 * hx, 384)], kc == 0,
                           (kc == MC - 1 or "pass2" in SKIP))
                if r1B is None:
                    r1B = emit_r1()
                if ni % 2 == 0:
                    stage4 = sp.tile([128, 2, P, H], bf16, tag="ostage")
                with nc.allow_low_precision(reason="out stored bf16"):
                    for hx in range(2):
                        nc.vector.tensor_tensor(
                            stage4[:, ni % 2, ds(6 * hx, 6), :],
                            po[hx].rearrange("t (p h) -> t p h", h=H),
                            r1B[:, ds(384 * hx, 384)].rearrange("t (p h) -> t p h", h=H),
                            Alu.add)
                if "out" not in SKIP and ni >= KT - 2:
                    eng = nc.scalar if ni % 2 == 0 else nc.gpsimd
                    eng.dma_start(
                        out=out_d[b][ni, :, :, :],
                        in_=stage4[:, ni % 2, :, :])
                elif "out" not in SKIP and ni % 2 == 1:
                    eng = nc.scalar if (ni // 2) % 2 == 0 else nc.gpsimd
                    eng.dma_start(
                        out=out_d[b][ds(ni - 1, 2), :, :, :].transpose([1, 0, 2, 3]),
                        in_=stage4[:, :, :, :])

    nc.compile()
    return nc


def _host_prep(inputs):
    import ml_dtypes
    bf = ml_dtypes.bfloat16
    e4 = ml_dtypes.float8_e4m3
    x = np.asarray(inputs["x"], dtype=np.float32)
    edge_index = np.asarray(inputs["edge_index"])
    g_w = np.asarray(inputs["g_norm_w"], dtype=np.float32)
    g_b = np.asarray(inputs["g_norm_b"], dtype=np.float32)
    t_w = np.asarray(inputs["t_norm_w"], dtype=np.float32)
    t_b = np.asarray(inputs["t_norm_b"], dtype=np.float32)
    conv_w = np.asarray(inputs["conv_w"], dtype=np.float32)
    conv_b = np.asarray(inputs["conv_b"], dtype=np.float32)

    # fast path requires LN affine params constant (true for this problem family)
    assert np.all(g_w == g_w.flat[0]) and np.all(t_w == t_w.flat[0]), \
        "non-constant LayerNorm weight not supported by this kernel"
    kg = float(g_w.flat[0])
    kt = float(t_w.flat[0])
    assert np.all(t_b == t_b.flat[0]), "non-constant t_norm_b not supported"
    kb = float(t_b.flat[0])

    src = edge_index[0].astype(np.int64)
    dst = edge_index[1].astype(np.int64)
    deg = np.zeros(N, np.float32)
    np.add.at(deg, dst, np.float32(1.0))
    dinv = np.where(deg > 0, 1.0 / np.sqrt(np.maximum(deg, 1.0)), 0.0).astype(np.float32)

    # keep only edges with nonzero weight (dinv[src] > 0; dst always has deg>=1)
    keep = dinv[src] > 0
    srck, dstk = src[keep], dst[keep]

    # S: integer edge counts, exact in fp8. Row = src, col = dst.
    Sf = np.zeros((N, N), np.float32)
    np.add.at(Sf, (srck, dstk), np.float32(1.0))
    s8 = np.ascontiguousarray(Sf.reshape(128, KP, 2, N)).astype(e4)

    # u1[dst] = sum_e dinv[src_e]; corr folded as (c*G + ncu*u1 [+v1]) * dd
    u1 = np.zeros(N, np.float32)
    np.add.at(u1, dstk, dinv[srck])
    ddr = (kg * dinv).astype(np.float32)
    u1t2 = np.ascontiguousarray(np.broadcast_to(u1, (128, N))).astype(np.float32)
    ddt2 = np.ascontiguousarray(np.broadcast_to(ddr, (128, N))).astype(np.float32)

    # v = A @ g_b (element-wise over h); v1 = v / dd  (guard dd == 0)
    has_v = bool(np.any(g_b != 0))
    if has_v:
        A = np.zeros((N, N), np.float32)
        A[dstk, srck] = 0.0
        np.add.at(A, (dstk, srck), (dinv[srck] * dinv[dstk]).astype(np.float32))
        v = A @ g_b          # [N, H]
        vt2 = np.empty((128, N), np.float32)
        vt2[:64] = v.T; vt2[64:] = v.T
        with np.errstate(divide="ignore", invalid="ignore"):
            v1t2 = np.where(ddt2 != 0, vt2 / ddt2, 0.0).astype(np.float32)
    else:
        v1t2 = np.zeros((1, 1), np.float32)  # unused

    # x scaled by dinv[src], split hi+lo fp8; plus raw bf16 for LN1 stats
    xb = np.ascontiguousarray(
        x.astype(bf).reshape(B, P, 128, KT, H).transpose(0, 2, 3, 1, 4))
    xs = x * dinv[None, None, :, None]
    xs = np.ascontiguousarray(
        xs.reshape(B, P, 128, KP, 2, H).transpose(0, 2, 3, 4, 1, 5))
    x8h = xs.astype(e4)
    x8l = (xs - x8h.astype(np.float32)).astype(e4)

    cwi = np.zeros((PH, PH), np.float32)
    for p in range(P):
        for q in range(P):
            w = conv_w[q, p] * kt
            idx = np.arange(H)
            cwi[p * H + idx, q * H + idx] = w
    cwi = cwi.astype(bf)

    r12 = np.zeros((P, PH), np.float32)
    for p in range(P):
        r12[p, p * H:(p + 1) * H] = 1.0
    bo = np.zeros((PH, P), np.float32)
    for p in range(P):
        bo[p * H:(p + 1) * H, p] = 1.0
    cwt = np.ascontiguousarray(conv_w.T * kt)
    cb = (conv_b + kb * conv_w.sum(axis=1)).astype(np.float32).reshape(P, 1)

    consts = {"s8": s8, "u1": u1t2, "dd": ddt2, "cwi": cwi,
              "r12": r12, "bo": bo, "cwt": cwt, "cb": cb}
    if has_v:
        consts["v1"] = v1t2
    return (xb, x8h, x8l), consts, has_v


def _unpack_out(arr):
    """[BL, KT(ni), 128, P, H] bf16 -> [BL, P, N, H] f32 with n = ni*128 + t."""
    return np.ascontiguousarray(
        arr.astype(np.float32).transpose(0, 3, 1, 2, 4).reshape(BL, P, N, H))


def kernel(**inputs):
    from concourse.bass_utils import run_bass_kernel_spmd

    (xb, x8h, x8l), consts, has_v = _host_prep(inputs)

    if ("nc", has_v) not in _CACHE:
        _CACHE[("nc", has_v)] = _build_program(has_v)
    nc = _CACHE[("nc", has_v)]

    in_maps = []
    for c in range(NCORES):
        sl = slice(c * BL, (c + 1) * BL)
        m = {"xb": np.ascontiguousarray(xb[sl]),
             "x8h": np.ascontiguousarray(x8h[sl]),
             "x8l": np.ascontiguousarray(x8l[sl])}
        m.update(consts)
        in_maps.append(m)

    res = run_bass_kernel_spmd(nc, in_maps, core_ids=list(range(NCORES)))
    out = np.empty((B, P, N, H), np.float32)
    for c in range(NCORES):
        out[c * BL:(c + 1) * BL] = _unpack_out(res.results[c]["out"])
    return out


# revision 22
# speedup vs baseline: 1.3135x; 1.0372x over previous
"""Trainium2 Bass kernel for nn_CondBlock (LayerNorm -> LightGCN conv -> LayerNorm -> 1x1 conv over P).

Self-contained: hardcoded shapes, host-side graph preprocessing, 8-core
data-parallel (over batch) SPMD execution via run_bass_kernel_spmd.

Algorithm (validated vs reference in fp32):
  per slice s=(b,p): LN1: h1 = c_s*(x - mu_s)*g_w + g_b, c_s = rsqrt(var_s+eps)
  conv:  A = D_dst S D_src with S integer edge counts (exact in fp8).
         Device matmul computes G = S^T @ (dinv_src * x) using fp8e4
         DoubleRow matmuls (hi+lo residual split of the scaled x, both
         accumulated in PSUM -> ~1e-3 precision at 2x bf16 PE rate).
         Evict: Z = (c*G + ncu*u1 [+ v1]) * dd, dd = kg*dinv_dst (per col).
  LN2 + P-mix folded:
         out_q = sum_p aa[q,p]*Z_p + r1[q],  aa[q,p] = conv_w[q,p]*c2_p*kt
         r1[q] = -sum_p aa[q,p]*mu2_p + kb*sum_p conv_w[q,p] + conv_b[q]
  Pass-2 matmul  out[n,(q,h)] = Z^T-tiles^T @ W (bf16), W = (conv_w (x) I_64)*c2*kt
"""

import numpy as np

B, P, N, H = 16, 12, 2048, 64
E = 16384
NCORES = 8
BL = B // NCORES      # batches per core
PH = P * H            # 768
MC = PH // 128        # 6 (p,h)-chunks of 128
KT = N // 128         # 16 node tiles
KP = 8                # DoubleRow src chunk pairs (256 nodes each)
FQW = 512             # dst-column chunk width for pass-1
FQ = N // FQW         # 4
NH = float(N * H)
EPS = 1e-5

_CACHE = {}


def _build_program(has_v=False):
    import os
    SKIP = set(filter(None, os.environ.get("K_SKIP", "").split(",")))
    from concourse import bass, bacc, tile, mybir
    from contextlib import ExitStack

    f32 = mybir.dt.float32
    bf16 = mybir.dt.bfloat16
    fp8 = mybir.dt.float8e4
    ds = bass.ds
    Alu = mybir.AluOpType
    Act = mybir.ActivationFunctionType
    DR = mybir.MatmulPerfMode.DoubleRow

    nc = bacc.Bacc("TRN2", target_bir_lowering=False, debug=False)

    xb_d = nc.dram_tensor("xb", [BL, 128, KT, P, H], bf16, kind="ExternalInput").ap()
    x8h_d = nc.dram_tensor("x8h", [BL, 128, KP, 2, P, H], fp8, kind="ExternalInput").ap()
    x8l_d = nc.dram_tensor("x8l", [BL, 128, KP, 2, P, H], fp8, kind="ExternalInput").ap()
    s8_d = nc.dram_tensor("s8", [128, KP, 2, N], fp8, kind="ExternalInput").ap()
    u1_d = nc.dram_tensor("u1", [128, N], bf16, kind="ExternalInput").ap()
    dd_d = nc.dram_tensor("dd", [128, N], bf16, kind="ExternalInput").ap()
    v1_d = nc.dram_tensor("v1", [128, N], f32, kind="ExternalInput").ap() if has_v else None
    cwi_d = nc.dram_tensor("cwi", [PH, PH], bf16, kind="ExternalInput").ap()
    r12_d = nc.dram_tensor("r12", [P, PH], f32, kind="ExternalInput").ap()
    bo_d = nc.dram_tensor("bo", [PH, P], f32, kind="ExternalInput").ap()
    cwt_d = nc.dram_tensor("cwt", [P, P], f32, kind="ExternalInput").ap()
    cb_d = nc.dram_tensor("cb", [P, 1], f32, kind="ExternalInput").ap()
    out_d = nc.dram_tensor("out", [BL, KT, 128, P, H], bf16, kind="ExternalOutput").ap()

    with tile.TileContext(nc) as tc, ExitStack() as ctx:
        cons = ctx.enter_context(tc.tile_pool(name="cons", bufs=1))
        xpool = ctx.enter_context(tc.tile_pool(name="xp", bufs=1))
        zpool = ctx.enter_context(tc.tile_pool(name="zp", bufs=1))
        wpool = ctx.enter_context(tc.tile_pool(name="wp", bufs=1))
        sp = ctx.enter_context(tc.tile_pool(name="sp", bufs=2))
        sml = ctx.enter_context(tc.tile_pool(name="sml", bufs=1))
        pp = ctx.enter_context(tc.tile_pool(name="pp", bufs=6, space="PSUM"))

        # ---- constants ----
        u1t = cons.tile([128, N], bf16, tag="u1t")
        ddt = cons.tile([128, N], bf16, tag="ddt")
        v1t = cons.tile([128, N], f32, tag="v1t") if has_v else None
        r12 = cons.tile([P, PH], f32, tag="r12")
        bo = cons.tile([128, MC, P], f32, tag="bo")
        cwt = cons.tile([P, P], f32, tag="cwt")
        cb = cons.tile([P, 1], f32, tag="cb")
        onesk = cons.tile([128, 1], bf16, tag="onesk")
        onesm = cons.tile([1, 128], f32, tag="onesm")
        def load_consts():
            nc.scalar.dma_start(out=u1t[:, :], in_=u1_d[:, :])
            nc.scalar.dma_start(out=ddt[:, :], in_=dd_d[:, :])
            if has_v:
                nc.scalar.dma_start(out=v1t[:, :], in_=v1_d[:, :])
            nc.scalar.dma_start(out=r12[:, :], in_=r12_d[:, :])
            nc.scalar.dma_start(out=bo[:, :, :], in_=bo_d.rearrange("(c t) p -> t c p", t=128))
            nc.scalar.dma_start(out=cwt[:, :], in_=cwt_d[:, :])
            nc.scalar.dma_start(out=cb[:, :], in_=cb_d[:, :])
        onesf = cons.tile([128, 1], f32, tag="onesf")
        nc.vector.memset(onesf[:, :], 1.0)
        nc.vector.tensor_copy(onesk[:, :], onesf[:, :])
        nc.vector.memset(onesm[:, :], 1.0)

        s8 = ctx.enter_context(tc.tile_pool(name="s8p", bufs=1)).tile(
            [128, KP, 2, N], fp8, tag="S8")

        def load_s8(kcs):
            for kc in kcs:
                nc.sync.dma_start(
                    out=s8[:, ds(2 * kc, 2), :, :],
                    in_=s8_d[:, ds(2 * kc, 2), :, :])

        def mm(out, lhsT, rhs, start, stop):
            nc.tensor.matmul(out, lhsT, rhs, start=start, stop=stop)

        def col12(row):
            """[1,12] sbuf row -> [12,1] sbuf col (via PE)."""
            ps = pp.tile([12, 1], f32, tag="ps")
            mm(ps[:, :], row, onesm[:, 0:1], True, True)
            col = sml.tile([12, 1], f32, tag=None)
            nc.vector.tensor_copy(col[:, :], ps[:, :])
            return col

        def expand12(col_sb, dst):
            """[12,1] sbuf col -> dst [128, MC] per-partition cols (c[p] replicated over h)."""
            for m in range(MC):
                ps = pp.tile([128, 1], f32, tag="ps")
                mm(ps[:, :], r12[:, ds(m * 128, 128)], col_sb, True, True)
                nc.vector.tensor_copy(dst[:, m:m + 1], ps[:, :])

        def emit_x8loads(b, X8h, X8l):
            for kh in range(4):
                nc.gpsimd.dma_start(out=X8h[:, ds(2 * kh, 2), :, :, :],
                                    in_=x8h_d[b][:, ds(2 * kh, 2), :, :, :])
                nc.gpsimd.dma_start(out=X8l[:, ds(2 * kh, 2), :, :, :],
                                    in_=x8l_d[b][:, ds(2 * kh, 2), :, :, :])

        def emit_xbload(b, X):
            for kh in range(4):
                nc.sync.dma_start(
                    out=X[:, ds(4 * kh, 4), :, :],
                    in_=xb_d[b][:, ds(4 * kh, 4), :, :])

        def emit_stats(b, X):
            """LN1 stats for batch b -> (c_col, ncu_col) [128, MC] tiles."""
            NKS = KT if "stats" not in SKIP else 1
            ps_s1 = pp.tile([1, 2, 512], f32, tag="ps2", name=f"ps_s1_{b}", bufs=1)
            for k in range(NKS):
                for hx in range(2):
                    mm(ps_s1[:, hx, 0:384], onesk[:, :],
                       X[:, k, 6 * hx:6 * hx + 6, :], k == 0, k == NKS - 1)
            s1row = sml.tile([1, PH], f32, tag="s1row")
            for hx in range(2):
                nc.vector.tensor_copy(s1row[:, ds(384 * hx, 384)], ps_s1[:, hx, 0:384])
            ps_q1 = pp.tile([1, 2, 512], f32, tag="ps2", name=f"ps_q1_{b}", bufs=1)
            for k in range(NKS):
                sqx = sp.tile([128, P, H], bf16, tag="sqx")
                nc.scalar.activation(sqx[:, :, :], X[:, k, :, :], Act.Square)
                for hx in range(2):
                    mm(ps_q1[:, hx, 0:384], onesk[:, :],
                       sqx[:, 6 * hx:6 * hx + 6, :], k == 0, k == NKS - 1)
            q1row = sml.tile([1, PH], f32, tag="q1row")
            for hx in range(2):
                nc.vector.tensor_copy(q1row[:, ds(384 * hx, 384)], ps_q1[:, hx, 0:384])
            s1p = sml.tile([1, P], f32, tag="s1p")
            q1p = sml.tile([1, P], f32, tag="q1p")
            with nc.allow_low_precision(reason="12-col reduce in f32"):
                nc.vector.tensor_reduce(s1p[:, :], s1row.rearrange("o (p h) -> o p h", h=H),
                                        mybir.AxisListType.X, Alu.add)
                nc.vector.tensor_reduce(q1p[:, :], q1row.rearrange("o (p h) -> o p h", h=H),
                                        mybir.AxisListType.X, Alu.add)
            s1c = col12(s1p[:, :])
            q1c = col12(q1p[:, :])
            # mu, var, c = rsqrt(var+eps), ncu = -c*mu   (all [12,1])
            mu = sml.tile([P, 1], f32, tag="mu")
            var = sml.tile([P, 1], f32, tag="var")
            tmp = sml.tile([P, 1], f32, tag="tmp")
            c12t = sml.tile([P, 1], f32, tag="c12t")
            ncu12 = sml.tile([P, 1], f32, tag="ncu12")
            nc.vector.tensor_scalar(mu[:, :], s1c[:, :], 1.0 / NH, None, Alu.mult)
            nc.vector.tensor_tensor(tmp[:, :], mu[:, :], mu[:, :], Alu.mult)
            nc.vector.scalar_tensor_tensor(var[:, :], q1c[:, :], 1.0 / NH, tmp[:, :],
                                           Alu.mult, Alu.subtract)
            nc.vector.tensor_scalar(var[:, :], var[:, :], EPS, None, Alu.add)
            nc.vector.reciprocal(tmp[:, :], var[:, :])
            nc.scalar.activation(c12t[:, :], tmp[:, :], Act.Sqrt)
            nc.vector.scalar_tensor_tensor(ncu12[:, :], c12t[:, :], -1.0, mu[:, :],
                                           Alu.mult, Alu.mult)
            c_col = sml.tile([128, MC], f32, tag="c_col", bufs=2)
            ncu_col = sml.tile([128, MC], f32, tag="ncu_col", bufs=2)
            expand12(c12t[:, :], c_col)
            expand12(ncu12[:, :], ncu_col)
            return c_col, ncu_col

        nxt = None
        for b in range(BL):
            X8h = xpool.tile([128, KP, 2, P, H], fp8, tag="X8h")
            X8l = xpool.tile([128, KP, 2, P, H], fp8, tag="X8l")
            if b == 0:
                X = xpool.tile([128, KT, P, H], bf16, tag="X", bufs=2)
                emit_xbload(0, X)
                load_s8([0])
                nc.gpsimd.dma_start(out=X8h[:, ds(0, 2), :, :, :],
                                    in_=x8h_d[0][:, ds(0, 2), :, :, :])
                nc.gpsimd.dma_start(out=X8l[:, ds(0, 2), :, :, :],
                                    in_=x8l_d[0][:, ds(0, 2), :, :, :])
                load_s8([1, 2, 3])
                load_consts()
                for kh in range(1, 4):
                    nc.gpsimd.dma_start(out=X8h[:, ds(2 * kh, 2), :, :, :],
                                        in_=x8h_d[0][:, ds(2 * kh, 2), :, :, :])
                    nc.gpsimd.dma_start(out=X8l[:, ds(2 * kh, 2), :, :, :],
                                        in_=x8l_d[0][:, ds(2 * kh, 2), :, :, :])
                c_col, ncu_col = emit_stats(0, X)
                Xn1 = xpool.tile([128, KT, P, H], bf16, tag="X", bufs=2)
                emit_xbload(1, Xn1)
            else:
                X, c_col, ncu_col = nxt
                emit_x8loads(b, X8h, X8l)
                if b + 1 < BL:
                    Xn1 = xpool.tile([128, KT, P, H], bf16, tag="X", bufs=2)
                    emit_xbload(b + 1, Xn1)

            # ---- W staging: DMA CWI now (scaled by c2 later) ----
            W = wpool.tile([128, MC, PH], bf16, tag="W")
            nc.scalar.dma_start(out=W[:, :, :], in_=cwi_d.rearrange("(c t) f -> t c f", t=128))

            # ---- pass-1 conv: G = S^T @ (dinv_src*x) via fp8 DoubleRow hi+lo ----
            Z = zpool.tile([128, MC, N], bf16, tag="Z")
            zs_slots = sml.tile([128, MC, FQ], f32, tag="zs")
            zq_slots = sml.tile([128, MC, FQ], f32, tag="zq")
            for fq in range(FQ):
                gps = [pp.tile([128, FQW], f32, tag="ps", name=f"gps_{b}_{fq}_{i}") for i in range(MC)]
                NKC = KP if "conv" not in SKIP else 1
                if fq == 0:
                    # kp-outer: consume s8/x8 chunks as their DMAs land
                    for kp in range(NKC):
                        for si, Xs in enumerate((X8h, X8l)):
                            for m in range(MC):
                                nc.tensor.matmul(
                                    gps[m][:, :], Xs[:, kp, :, 2 * m:2 * m + 2, :],
                                    s8[:, kp, :, ds(fq * FQW, FQW)],
                                    start=(kp == 0 and si == 0),
                                    stop=(kp == NKC - 1 and si == 1),
                                    perf_mode=DR)
                else:
                    # m-outer: finish chunk m early so its evict pipelines
                    # under the remaining matmuls
                    for m in range(MC):
                        for kp in range(NKC):
                            for si, Xs in enumerate((X8h, X8l)):
                                nc.tensor.matmul(
                                    gps[m][:, :], Xs[:, kp, :, 2 * m:2 * m + 2, :],
                                    s8[:, kp, :, ds(fq * FQW, FQW)],
                                    start=(kp == 0 and si == 0),
                                    stop=(kp == NKC - 1 and si == 1),
                                    perf_mode=DR)
                for m in range(MC if "evict" not in SKIP else 0):
                    corr = sp.tile([128, FQW], f32, tag="corr")
                    if has_v:
                        nc.vector.scalar_tensor_tensor(
                            corr[:, :], u1t[:, ds(fq * FQW, FQW)], ncu_col[:, m:m + 1],
                            v1t[:, ds(fq * FQW, FQW)], Alu.mult, Alu.add)
                    else:
                        nc.scalar.activation(corr[:, :], u1t[:, ds(fq * FQW, FQW)],
                                             Act.Copy, scale=ncu_col[:, m:m + 1])
                    zt = sp.tile([128, FQW], f32, tag="zt")
                    nc.vector.scalar_tensor_tensor(
                        zt[:, :], gps[m][:, :], c_col[:, m:m + 1],
                        corr[:, :], Alu.mult, Alu.add)
                    with nc.allow_low_precision(reason="Z stored bf16 for pass-2"):
                        nc.vector.scalar_tensor_tensor(
                            Z[:, m, ds(fq * FQW, FQW)], zt[:, :], 1.0,
                            ddt[:, ds(fq * FQW, FQW)], Alu.mult, Alu.mult,
                            accum_out=zs_slots[:, m, fq:fq + 1])
                    sqz = sp.tile([128, FQW], f32, tag="sqz")
                    zsl = Z[:, m, ds(fq * FQW, FQW)]
                    nc.scalar.activation(sqz[:, :], zsl, Act.Square,
                                         accum_out=zq_slots[:, m, fq:fq + 1])

            # ---- LN2 stats ----
            zs6 = sml.tile([128, MC], f32, tag="zs6")
            zq6 = sml.tile([128, MC], f32, tag="zq6")
            with nc.allow_low_precision(reason="4-col reduce in f32"):
                nc.vector.tensor_reduce(zs6[:, :], zs_slots[:, :, :], mybir.AxisListType.X, Alu.add)
                nc.vector.tensor_reduce(zq6[:, :], zq_slots[:, :, :], mybir.AxisListType.X, Alu.add)
            ps_s2 = pp.tile([P, 1], f32, tag="ps")
            ps_q2 = pp.tile([P, 1], f32, tag="ps")
            for m in range(MC):
                mm(ps_s2[:, :], bo[:, m, :], zs6[:, m:m + 1], m == 0, m == MC - 1)
                mm(ps_q2[:, :], bo[:, m, :], zq6[:, m:m + 1], m == 0, m == MC - 1)
            s2c = sml.tile([P, 1], f32, tag="s2c")
            q2c = sml.tile([P, 1], f32, tag="q2c")
            nc.vector.tensor_copy(s2c[:, :], ps_s2[:, :])
            nc.vector.tensor_copy(q2c[:, :], ps_q2[:, :])
            mu2 = sml.tile([P, 1], f32, tag="mu2")
            var2 = sml.tile([P, 1], f32, tag="var2")
            tmp2 = sml.tile([P, 1], f32, tag="tmp2")
            c2t = sml.tile([P, 1], f32, tag="c2t")
            nc.vector.tensor_scalar(mu2[:, :], s2c[:, :], 1.0 / NH, None, Alu.mult)
            nc.vector.tensor_tensor(tmp2[:, :], mu2[:, :], mu2[:, :], Alu.mult)
            nc.vector.scalar_tensor_tensor(var2[:, :], q2c[:, :], 1.0 / NH, tmp2[:, :],
                                           Alu.mult, Alu.subtract)
            nc.vector.tensor_scalar(var2[:, :], var2[:, :], EPS, None, Alu.add)
            nc.vector.reciprocal(tmp2[:, :], var2[:, :])
            nc.scalar.activation(c2t[:, :], tmp2[:, :], Act.Sqrt)
            c2_col = sml.tile([128, MC], f32, tag="c2col")
            expand12(c2t[:, :], c2_col)
            # W = CWI * c2 (per-partition scale)
            with nc.allow_low_precision(reason="W bf16"):
                for m in range(MC):
                    eng = nc.gpsimd if m % 2 else nc.vector
                    eng.tensor_scalar(W[:, m, :], W[:, m, :], c2_col[:, m:m + 1],
                                      None, Alu.mult)

            def emit_r1():
                # r1[q] = cb[q] - sum_p A1[p,q]*mu2[p],  A1 = cwt*c2
                a1 = sml.tile([P, P], f32, tag="a1")
                nc.vector.tensor_scalar(a1[:, :], cwt[:, :], c2t[:, :], None, Alu.mult)
                ps_k1 = pp.tile([P, 1], f32, tag="ps2", bufs=1, name="ps_k1_r1")
                mm(ps_k1[:, :], a1[:, :], mu2[:, :], True, True)
                r1c = sml.tile([P, 1], f32, tag="r1c")
                nc.vector.tensor_tensor(r1c[:, :], cb[:, :], ps_k1[:, :], Alu.subtract)
                r1row = sml.tile([1, PH], f32, tag="r1row")
                r1B = sml.tile([128, PH], f32, tag="r1B")
                for hx in range(2):
                    psr = pp.tile([1, 384], f32, tag="ps2", bufs=1, name=f"psr_{hx}")
                    mm(psr[:, :], r1c[:, :], r12[:, ds(384 * hx, 384)], True, True)
                    nc.vector.tensor_copy(r1row[:, ds(384 * hx, 384)], psr[:, :])
                for hx in range(2):
                    psb = pp.tile([128, 384], f32, tag="ps2", bufs=1, name=f"psb_{hx}")
                    mm(psb[:, :], onesm[:, :], r1row[:, ds(384 * hx, 384)], True, True)
                    nc.vector.tensor_copy(r1B[:, ds(384 * hx, 384)], psb[:, :])
                return r1B

            # ---- prefetch next batch: X loads + LN1 stats before pass-2 ----
            if b + 1 < BL:
                cc_n, nc_n = emit_stats(b + 1, Xn1)
                nxt = (Xn1, cc_n, nc_n)

            # ---- pass-2: out[n, (q,h)] = sum_c Z[:, c, n]^T @ W[:, c, :] ----
            r1B = None
            for ni in range(KT):
                po = [pp.tile([128, 384], f32, tag="ps", name=f"po_{b}_{ni}_{i}") for i in range(2)]
                for kc in range(MC if "pass2" not in SKIP else 1):
                    for hx in range(2):
                        mm(po[hx][:, :], Z[:, kc, ds(ni * 128, 128)],
                           W[:, kc, ds(384 * hx, 384)], kc == 0,
                           (kc == MC - 1 or "pass2" in SKIP))
                if r1B is None:
                    r1B = emit_r1()
                if ni % 2 == 0:
                    stage4 = sp.tile([128, 2, P, H], bf16, tag="ostage")
                with nc.allow_low_precision(reason="out stored bf16"):
                    for hx in range(2):
                        nc.vector.tensor_tensor(
                            stage4[:, ni % 2, ds(6 * hx, 6), :],
                            po[hx].rearrange("t (p h) -> t p h", h=H),
                            r1B[:, ds(384 * hx, 384)].rearrange("t (p h) -> t p h", h=H),
                            Alu.add)
                if "out" not in SKIP and ni >= KT - 2:
                    eng = nc.scalar if ni % 2 == 0 else nc.gpsimd
                    eng.dma_start(
                        out=out_d[b][ni, :, :, :],
                        in_=stage4[:, ni % 2, :, :])
                elif "out" not in SKIP and ni % 2 == 1:
                    eng = nc.scalar if (ni // 2) % 2 == 0 else nc.gpsimd
                    eng.dma_start(
                        out=out_d[b][ds(ni - 1, 2), :, :, :].transpose([1, 0, 2, 3]),
                        in_=stage4[:, :, :, :])

    nc.compile()
    return nc


def _host_prep(inputs):
    import ml_dtypes
    bf = ml_dtypes.bfloat16
    e4 = ml_dtypes.float8_e4m3
    x = np.asarray(inputs["x"], dtype=np.float32)
    edge_index = np.asarray(inputs["edge_index"])
    g_w = np.asarray(inputs["g_norm_w"], dtype=np.float32)
    g_b = np.asarray(inputs["g_norm_b"], dtype=np.float32)
    t_w = np.asarray(inputs["t_norm_w"], dtype=np.float32)
    t_b = np.asarray(inputs["t_norm_b"], dtype=np.float32)
    conv_w = np.asarray(inputs["conv_w"], dtype=np.float32)
    conv_b = np.asarray(inputs["conv_b"], dtype=np.float32)

    # fast path requires LN affine params constant (true for this problem family)
    assert np.all(g_w == g_w.flat[0]) and np.all(t_w == t_w.flat[0]), \
        "non-constant LayerNorm weight not supported by this kernel"
    kg = float(g_w.flat[0])
    kt = float(t_w.flat[0])
    assert np.all(t_b == t_b.flat[0]), "non-constant t_norm_b not supported"
    kb = float(t_b.flat[0])

    src = edge_index[0].astype(np.int64)
    dst = edge_index[1].astype(np.int64)
    deg = np.zeros(N, np.float32)
    np.add.at(deg, dst, np.float32(1.0))
    dinv = np.where(deg > 0, 1.0 / np.sqrt(np.maximum(deg, 1.0)), 0.0).astype(np.float32)

    # keep only edges with nonzero weight (dinv[src] > 0; dst always has deg>=1)
    keep = dinv[src] > 0
    srck, dstk = src[keep], dst[keep]

    # S: integer edge counts, exact in fp8. Row = src, col = dst.
    Sf = np.zeros((N, N), np.float32)
    np.add.at(Sf, (srck, dstk), np.float32(1.0))
    s8 = np.ascontiguousarray(Sf.reshape(128, KP, 2, N)).astype(e4)

    # u1[dst] = sum_e dinv[src_e]; corr folded as (c*G + ncu*u1 [+v1]) * dd
    u1 = np.zeros(N, np.float32)
    np.add.at(u1, dstk, dinv[srck])
    ddr = (kg * dinv).astype(np.float32)
    u1t2 = np.ascontiguousarray(np.broadcast_to(u1, (128, N))).astype(bf)
    ddt2 = np.ascontiguousarray(np.broadcast_to(ddr, (128, N))).astype(bf)

    # v = A @ g_b (element-wise over h); v1 = v / dd  (guard dd == 0)
    has_v = bool(np.any(g_b != 0))
    if has_v:
        A = np.zeros((N, N), np.float32)
        A[dstk, srck] = 0.0
        np.add.at(A, (dstk, srck), (dinv[srck] * dinv[dstk]).astype(np.float32))
        v = A @ g_b          # [N, H]
        vt2 = np.empty((128, N), np.float32)
        vt2[:64] = v.T; vt2[64:] = v.T
        dd32 = ddt2.astype(np.float32)
        with np.errstate(divide="ignore", invalid="ignore"):
            v1t2 = np.where(dd32 != 0, vt2 / dd32, 0.0).astype(np.float32)
    else:
        v1t2 = np.zeros((1, 1), np.float32)  # unused

    # x scaled by dinv[src], split hi+lo fp8; plus raw bf16 for LN1 stats
    xb = np.ascontiguousarray(
        x.astype(bf).reshape(B, P, 128, KT, H).transpose(0, 2, 3, 1, 4))
    xs = x * dinv[None, None, :, None]
    xs = np.ascontiguousarray(
        xs.reshape(B, P, 128, KP, 2, H).transpose(0, 2, 3, 4, 1, 5))
    x8h = xs.astype(e4)
    x8l = (xs - x8h.astype(np.float32)).astype(e4)

    cwi = np.zeros((PH, PH), np.float32)
    for p in range(P):
        for q in range(P):
            w = conv_w[q, p] * kt
            idx = np.arange(H)
            cwi[p * H + idx, q * H + idx] = w
    cwi = cwi.astype(bf)

    r12 = np.zeros((P, PH), np.float32)
    for p in range(P):
        r12[p, p * H:(p + 1) * H] = 1.0
    bo = np.zeros((PH, P), np.float32)
    for p in range(P):
        bo[p * H:(p + 1) * H, p] = 1.0
    cwt = np.ascontiguousarray(conv_w.T * kt)
    cb = (conv_b + kb * conv_w.sum(axis=1)).astype(np.float32).reshape(P, 1)

    consts = {"s8": s8, "u1": u1t2, "dd": ddt2, "cwi": cwi,
              "r12": r12, "bo": bo, "cwt": cwt, "cb": cb}
    if has_v:
        consts["v1"] = v1t2
    return (xb, x8h, x8l), consts, has_v


def _unpack_out(arr):
    """[BL, KT(ni), 128, P, H] bf16 -> [BL, P, N, H] f32 with n = ni*128 + t."""
    return np.ascontiguousarray(
        arr.astype(np.float32).transpose(0, 3, 1, 2, 4).reshape(BL, P, N, H))


def kernel(**inputs):
    from concourse.bass_utils import run_bass_kernel_spmd

    (xb, x8h, x8l), consts, has_v = _host_prep(inputs)

    if ("nc", has_v) not in _CACHE:
        _CACHE[("nc", has_v)] = _build_program(has_v)
    nc = _CACHE[("nc", has_v)]

    in_maps = []
    for c in range(NCORES):
        sl = slice(c * BL, (c + 1) * BL)
        m = {"xb": np.ascontiguousarray(xb[sl]),
             "x8h": np.ascontiguousarray(x8h[sl]),
             "x8l": np.ascontiguousarray(x8l[sl])}
        m.update(consts)
        in_maps.append(m)

    res = run_bass_kernel_spmd(nc, in_maps, core_ids=list(range(NCORES)))
    out = np.empty((B, P, N, H), np.float32)
    for c in range(NCORES):
        out[c * BL:(c + 1) * BL] = _unpack_out(res.results[c]["out"])
    return out


# revision 23
# speedup vs baseline: 1.3166x; 1.0024x over previous
"""Trainium2 Bass kernel for nn_CondBlock (LayerNorm -> LightGCN conv -> LayerNorm -> 1x1 conv over P).

Self-contained: hardcoded shapes, host-side graph preprocessing, 8-core
data-parallel (over batch) SPMD execution via run_bass_kernel_spmd.

Algorithm (validated vs reference in fp32):
  per slice s=(b,p): LN1: h1 = c_s*(x - mu_s)*g_w + g_b, c_s = rsqrt(var_s+eps)
  conv:  A = D_dst S D_src with S integer edge counts (exact in fp8).
         Device matmul computes G = S^T @ (dinv_src * x) using fp8e4
         DoubleRow matmuls (hi+lo residual split of the scaled x, both
         accumulated in PSUM -> ~1e-3 precision at 2x bf16 PE rate).
         Evict: Z = (c*G + ncu*u1 [+ v1]) * dd, dd = kg*dinv_dst (per col).
  LN2 + P-mix folded:
         out_q = sum_p aa[q,p]*Z_p + r1[q],  aa[q,p] = conv_w[q,p]*c2_p*kt
         r1[q] = -sum_p aa[q,p]*mu2_p + kb*sum_p conv_w[q,p] + conv_b[q]
  Pass-2 matmul  out[n,(q,h)] = Z^T-tiles^T @ W (bf16), W = (conv_w (x) I_64)*c2*kt
"""

import numpy as np

B, P, N, H = 16, 12, 2048, 64
E = 16384
NCORES = 8
BL = B // NCORES      # batches per core
PH = P * H            # 768
MC = PH // 128        # 6 (p,h)-chunks of 128
KT = N // 128         # 16 node tiles
KP = 8                # DoubleRow src chunk pairs (256 nodes each)
FQW = 512             # dst-column chunk width for pass-1
FQ = N // FQW         # 4
NH = float(N * H)
EPS = 1e-5

_CACHE = {}


def _build_program(has_v=False):
    import os
    SKIP = set(filter(None, os.environ.get("K_SKIP", "").split(",")))
    from concourse import bass, bacc, tile, mybir
    from contextlib import ExitStack

    f32 = mybir.dt.float32
    bf16 = mybir.dt.bfloat16
    fp8 = mybir.dt.float8e4
    ds = bass.ds
    Alu = mybir.AluOpType
    Act = mybir.ActivationFunctionType
    DR = mybir.MatmulPerfMode.DoubleRow

    nc = bacc.Bacc("TRN2", target_bir_lowering=False, debug=False)

    xb_d = nc.dram_tensor("xb", [BL, 128, KT, P, H], bf16, kind="ExternalInput").ap()
    x8h_d = nc.dram_tensor("x8h", [BL, 128, KP, 2, P, H], fp8, kind="ExternalInput").ap()
    x8l_d = nc.dram_tensor("x8l", [BL, 128, KP, 2, P, H], fp8, kind="ExternalInput").ap()
    s8_d = nc.dram_tensor("s8", [128, KP, 2, N], fp8, kind="ExternalInput").ap()
    u1_d = nc.dram_tensor("u1", [128, N], bf16, kind="ExternalInput").ap()
    dd_d = nc.dram_tensor("dd", [128, N], bf16, kind="ExternalInput").ap()
    v1_d = nc.dram_tensor("v1", [128, N], f32, kind="ExternalInput").ap() if has_v else None
    cwi_d = nc.dram_tensor("cwi", [PH, PH], bf16, kind="ExternalInput").ap()
    r12_d = nc.dram_tensor("r12", [P, PH], f32, kind="ExternalInput").ap()
    bo_d = nc.dram_tensor("bo", [PH, P], f32, kind="ExternalInput").ap()
    cwt_d = nc.dram_tensor("cwt", [P, P], f32, kind="ExternalInput").ap()
    cb_d = nc.dram_tensor("cb", [P, 1], f32, kind="ExternalInput").ap()
    out_d = nc.dram_tensor("out", [BL, KT, 128, P, H], bf16, kind="ExternalOutput").ap()

    with tile.TileContext(nc) as tc, ExitStack() as ctx:
        cons = ctx.enter_context(tc.tile_pool(name="cons", bufs=1))
        xpool = ctx.enter_context(tc.tile_pool(name="xp", bufs=1))
        zpool = ctx.enter_context(tc.tile_pool(name="zp", bufs=1))
        wpool = ctx.enter_context(tc.tile_pool(name="wp", bufs=1))
        sp = ctx.enter_context(tc.tile_pool(name="sp", bufs=2))
        sml = ctx.enter_context(tc.tile_pool(name="sml", bufs=1))
        pp = ctx.enter_context(tc.tile_pool(name="pp", bufs=6, space="PSUM"))

        # ---- constants ----
        u1t = cons.tile([128, N], bf16, tag="u1t")
        ddt = cons.tile([128, N], bf16, tag="ddt")
        v1t = cons.tile([128, N], f32, tag="v1t") if has_v else None
        r12 = cons.tile([P, PH], f32, tag="r12")
        bo = cons.tile([128, MC, P], f32, tag="bo")
        cwt = cons.tile([P, P], f32, tag="cwt")
        cb = cons.tile([P, 1], f32, tag="cb")
        onesk = cons.tile([128, 1], bf16, tag="onesk")
        onesm = cons.tile([1, 128], f32, tag="onesm")
        def load_consts():
            nc.scalar.dma_start(out=u1t[:, :], in_=u1_d[:, :])
            nc.scalar.dma_start(out=ddt[:, :], in_=dd_d[:, :])
            if has_v:
                nc.scalar.dma_start(out=v1t[:, :], in_=v1_d[:, :])
            nc.scalar.dma_start(out=r12[:, :], in_=r12_d[:, :])
            nc.scalar.dma_start(out=bo[:, :, :], in_=bo_d.rearrange("(c t) p -> t c p", t=128))
            nc.scalar.dma_start(out=cwt[:, :], in_=cwt_d[:, :])
            nc.scalar.dma_start(out=cb[:, :], in_=cb_d[:, :])
        onesf = cons.tile([128, 1], f32, tag="onesf")
        nc.vector.memset(onesf[:, :], 1.0)
        nc.vector.tensor_copy(onesk[:, :], onesf[:, :])
        nc.vector.memset(onesm[:, :], 1.0)

        s8 = ctx.enter_context(tc.tile_pool(name="s8p", bufs=1)).tile(
            [128, KP, 2, N], fp8, tag="S8")

        def load_s8(kcs):
            for kc in kcs:
                nc.sync.dma_start(
                    out=s8[:, ds(2 * kc, 2), :, :],
                    in_=s8_d[:, ds(2 * kc, 2), :, :])

        def mm(out, lhsT, rhs, start, stop):
            nc.tensor.matmul(out, lhsT, rhs, start=start, stop=stop)

        def col12(row):
            """[1,12] sbuf row -> [12,1] sbuf col (via PE)."""
            ps = pp.tile([12, 1], f32, tag="ps")
            mm(ps[:, :], row, onesm[:, 0:1], True, True)
            col = sml.tile([12, 1], f32, tag=None)
            nc.vector.tensor_copy(col[:, :], ps[:, :])
            return col

        def expand12(col_sb, dst):
            """[12,1] sbuf col -> dst [128, MC] per-partition cols (c[p] replicated over h)."""
            for m in range(MC):
                ps = pp.tile([128, 1], f32, tag="ps")
                mm(ps[:, :], r12[:, ds(m * 128, 128)], col_sb, True, True)
                nc.vector.tensor_copy(dst[:, m:m + 1], ps[:, :])

        def emit_x8loads(b, X8h, X8l):
            for kh in range(4):
                nc.gpsimd.dma_start(out=X8h[:, ds(2 * kh, 2), :, :, :],
                                    in_=x8h_d[b][:, ds(2 * kh, 2), :, :, :])
                nc.gpsimd.dma_start(out=X8l[:, ds(2 * kh, 2), :, :, :],
                                    in_=x8l_d[b][:, ds(2 * kh, 2), :, :, :])

        def emit_xbload(b, X):
            for kh in range(4):
                nc.sync.dma_start(
                    out=X[:, ds(4 * kh, 4), :, :],
                    in_=xb_d[b][:, ds(4 * kh, 4), :, :])

        def emit_stats(b, X):
            """LN1 stats for batch b -> (c_col, ncu_col) [128, MC] tiles."""
            NKS = KT if "stats" not in SKIP else 1
            ps_s1 = pp.tile([1, 2, 512], f32, tag="ps2", name=f"ps_s1_{b}", bufs=1)
            for k in range(NKS):
                for hx in range(2):
                    mm(ps_s1[:, hx, 0:384], onesk[:, :],
                       X[:, k, 6 * hx:6 * hx + 6, :], k == 0, k == NKS - 1)
            s1row = sml.tile([1, PH], f32, tag="s1row")
            for hx in range(2):
                nc.vector.tensor_copy(s1row[:, ds(384 * hx, 384)], ps_s1[:, hx, 0:384])
            ps_q1 = pp.tile([1, 2, 512], f32, tag="ps2", name=f"ps_q1_{b}", bufs=1)
            for k in range(NKS):
                sqx = sp.tile([128, P, H], bf16, tag="sqx")
                nc.scalar.activation(sqx[:, :, :], X[:, k, :, :], Act.Square)
                for hx in range(2):
                    mm(ps_q1[:, hx, 0:384], onesk[:, :],
                       sqx[:, 6 * hx:6 * hx + 6, :], k == 0, k == NKS - 1)
            q1row = sml.tile([1, PH], f32, tag="q1row")
            for hx in range(2):
                nc.vector.tensor_copy(q1row[:, ds(384 * hx, 384)], ps_q1[:, hx, 0:384])
            s1p = sml.tile([1, P], f32, tag="s1p")
            q1p = sml.tile([1, P], f32, tag="q1p")
            with nc.allow_low_precision(reason="12-col reduce in f32"):
                nc.vector.tensor_reduce(s1p[:, :], s1row.rearrange("o (p h) -> o p h", h=H),
                                        mybir.AxisListType.X, Alu.add)
                nc.vector.tensor_reduce(q1p[:, :], q1row.rearrange("o (p h) -> o p h", h=H),
                                        mybir.AxisListType.X, Alu.add)
            s1c = col12(s1p[:, :])
            q1c = col12(q1p[:, :])
            # mu, var, c = rsqrt(var+eps), ncu = -c*mu   (all [12,1])
            mu = sml.tile([P, 1], f32, tag="mu")
            var = sml.tile([P, 1], f32, tag="var")
            tmp = sml.tile([P, 1], f32, tag="tmp")
            c12t = sml.tile([P, 1], f32, tag="c12t")
            ncu12 = sml.tile([P, 1], f32, tag="ncu12")
            nc.vector.tensor_scalar(mu[:, :], s1c[:, :], 1.0 / NH, None, Alu.mult)
            nc.vector.tensor_tensor(tmp[:, :], mu[:, :], mu[:, :], Alu.mult)
            nc.vector.scalar_tensor_tensor(var[:, :], q1c[:, :], 1.0 / NH, tmp[:, :],
                                           Alu.mult, Alu.subtract)
            nc.vector.tensor_scalar(var[:, :], var[:, :], EPS, None, Alu.add)
            nc.vector.reciprocal(tmp[:, :], var[:, :])
            nc.scalar.activation(c12t[:, :], tmp[:, :], Act.Sqrt)
            nc.vector.scalar_tensor_tensor(ncu12[:, :], c12t[:, :], -1.0, mu[:, :],
                                           Alu.mult, Alu.mult)
            c_col = sml.tile([128, MC], f32, tag="c_col", bufs=2)
            ncu_col = sml.tile([128, MC], f32, tag="ncu_col", bufs=2)
            expand12(c12t[:, :], c_col)
            expand12(ncu12[:, :], ncu_col)
            return c_col, ncu_col

        nxt = None
        for b in range(BL):
            X8h = xpool.tile([128, KP, 2, P, H], fp8, tag="X8h")
            X8l = xpool.tile([128, KP, 2, P, H], fp8, tag="X8l")
            if b == 0:
                X = xpool.tile([128, KT, P, H], bf16, tag="X", bufs=2)
                emit_xbload(0, X)
                load_s8([0])
                nc.gpsimd.dma_start(out=X8h[:, ds(0, 2), :, :, :],
                                    in_=x8h_d[0][:, ds(0, 2), :, :, :])
                nc.gpsimd.dma_start(out=X8l[:, ds(0, 2), :, :, :],
                                    in_=x8l_d[0][:, ds(0, 2), :, :, :])
                load_consts()
                load_s8([1, 2, 3])
                for kh in range(1, 4):
                    nc.gpsimd.dma_start(out=X8h[:, ds(2 * kh, 2), :, :, :],
                                        in_=x8h_d[0][:, ds(2 * kh, 2), :, :, :])
                    nc.gpsimd.dma_start(out=X8l[:, ds(2 * kh, 2), :, :, :],
                                        in_=x8l_d[0][:, ds(2 * kh, 2), :, :, :])
                c_col, ncu_col = emit_stats(0, X)
                Xn1 = xpool.tile([128, KT, P, H], bf16, tag="X", bufs=2)
                emit_xbload(1, Xn1)
            else:
                X, c_col, ncu_col = nxt
                emit_x8loads(b, X8h, X8l)
                if b + 1 < BL:
                    Xn1 = xpool.tile([128, KT, P, H], bf16, tag="X", bufs=2)
                    emit_xbload(b + 1, Xn1)

            # ---- W staging: DMA CWI now (scaled by c2 later) ----
            W = wpool.tile([128, MC, PH], bf16, tag="W")
            nc.scalar.dma_start(out=W[:, :, :], in_=cwi_d.rearrange("(c t) f -> t c f", t=128))

            # ---- pass-1 conv: G = S^T @ (dinv_src*x) via fp8 DoubleRow hi+lo ----
            Z = zpool.tile([128, MC, N], bf16, tag="Z")
            zs_slots = sml.tile([128, MC, FQ], f32, tag="zs")
            zq_slots = sml.tile([128, MC, FQ], f32, tag="zq")
            for fq in range(FQ):
                gps = [pp.tile([128, FQW], f32, tag="ps", name=f"gps_{b}_{fq}_{i}") for i in range(MC)]
                NKC = KP if "conv" not in SKIP else 1
                if b == 0 and fq == 0:
                    # kp-outer: consume s8/x8 chunks as their DMAs land
                    for kp in range(NKC):
                        for si, Xs in enumerate((X8h, X8l)):
                            for m in range(MC):
                                nc.tensor.matmul(
                                    gps[m][:, :], Xs[:, kp, :, 2 * m:2 * m + 2, :],
                                    s8[:, kp, :, ds(fq * FQW, FQW)],
                                    start=(kp == 0 and si == 0),
                                    stop=(kp == NKC - 1 and si == 1),
                                    perf_mode=DR)
                else:
                    # m-outer: finish chunk m early so its evict pipelines
                    # under the remaining matmuls
                    for m in range(MC):
                        for kp in range(NKC):
                            for si, Xs in enumerate((X8h, X8l)):
                                nc.tensor.matmul(
                                    gps[m][:, :], Xs[:, kp, :, 2 * m:2 * m + 2, :],
                                    s8[:, kp, :, ds(fq * FQW, FQW)],
                                    start=(kp == 0 and si == 0),
                                    stop=(kp == NKC - 1 and si == 1),
                                    perf_mode=DR)
                for m in range(MC if "evict" not in SKIP else 0):
                    corr = sp.tile([128, FQW], f32, tag="corr")
                    if has_v:
                        nc.vector.scalar_tensor_tensor(
                            corr[:, :], u1t[:, ds(fq * FQW, FQW)], ncu_col[:, m:m + 1],
                            v1t[:, ds(fq * FQW, FQW)], Alu.mult, Alu.add)
                    else:
                        nc.scalar.activation(corr[:, :], u1t[:, ds(fq * FQW, FQW)],
                                             Act.Copy, scale=ncu_col[:, m:m + 1])
                    zt = sp.tile([128, FQW], f32, tag="zt")
                    nc.vector.scalar_tensor_tensor(
                        zt[:, :], gps[m][:, :], c_col[:, m:m + 1],
                        corr[:, :], Alu.mult, Alu.add)
                    with nc.allow_low_precision(reason="Z stored bf16 for pass-2"):
                        nc.vector.scalar_tensor_tensor(
                            Z[:, m, ds(fq * FQW, FQW)], zt[:, :], 1.0,
                            ddt[:, ds(fq * FQW, FQW)], Alu.mult, Alu.mult,
                            accum_out=zs_slots[:, m, fq:fq + 1])
                    sqz = sp.tile([128, FQW], f32, tag="sqz")
                    zsl = Z[:, m, ds(fq * FQW, FQW)]
                    nc.scalar.activation(sqz[:, :], zsl, Act.Square,
                                         accum_out=zq_slots[:, m, fq:fq + 1])

            # ---- LN2 stats ----
            zs6 = sml.tile([128, MC], f32, tag="zs6")
            zq6 = sml.tile([128, MC], f32, tag="zq6")
            with nc.allow_low_precision(reason="4-col reduce in f32"):
                nc.vector.tensor_reduce(zs6[:, :], zs_slots[:, :, :], mybir.AxisListType.X, Alu.add)
                nc.vector.tensor_reduce(zq6[:, :], zq_slots[:, :, :], mybir.AxisListType.X, Alu.add)
            ps_s2 = pp.tile([P, 1], f32, tag="ps")
            ps_q2 = pp.tile([P, 1], f32, tag="ps")
            for m in range(MC):
                mm(ps_s2[:, :], bo[:, m, :], zs6[:, m:m + 1], m == 0, m == MC - 1)
                mm(ps_q2[:, :], bo[:, m, :], zq6[:, m:m + 1], m == 0, m == MC - 1)
            s2c = sml.tile([P, 1], f32, tag="s2c")
            q2c = sml.tile([P, 1], f32, tag="q2c")
            nc.vector.tensor_copy(s2c[:, :], ps_s2[:, :])
            nc.vector.tensor_copy(q2c[:, :], ps_q2[:, :])
            mu2 = sml.tile([P, 1], f32, tag="mu2")
            var2 = sml.tile([P, 1], f32, tag="var2")
            tmp2 = sml.tile([P, 1], f32, tag="tmp2")
            c2t = sml.tile([P, 1], f32, tag="c2t")
            nc.vector.tensor_scalar(mu2[:, :], s2c[:, :], 1.0 / NH, None, Alu.mult)
            nc.vector.tensor_tensor(tmp2[:, :], mu2[:, :], mu2[:, :], Alu.mult)
            nc.vector.scalar_tensor_tensor(var2[:, :], q2c[:, :], 1.0 / NH, tmp2[:, :],
                                           Alu.mult, Alu.subtract)
            nc.vector.tensor_scalar(var2[:, :], var2[:, :], EPS, None, Alu.add)
            nc.vector.reciprocal(tmp2[:, :], var2[:, :])
            nc.scalar.activation(c2t[:, :], tmp2[:, :], Act.Sqrt)
            c2_col = sml.tile([128, MC], f32, tag="c2col")
            expand12(c2t[:, :], c2_col)
            # W = CWI * c2 (per-partition scale)
            with nc.allow_low_precision(reason="W bf16"):
                for m in range(MC):
                    eng = nc.gpsimd if m % 2 else nc.vector
                    eng.tensor_scalar(W[:, m, :], W[:, m, :], c2_col[:, m:m + 1],
                                      None, Alu.mult)

            def emit_r1():
                # r1[q] = cb[q] - sum_p A1[p,q]*mu2[p],  A1 = cwt*c2
                a1 = sml.tile([P, P], f32, tag="a1")
                nc.vector.tensor_scalar(a1[:, :], cwt[:, :], c2t[:, :], None, Alu.mult)
                ps_k1 = pp.tile([P, 1], f32, tag="ps2", bufs=1, name="ps_k1_r1")
                mm(ps_k1[:, :], a1[:, :], mu2[:, :], True, True)
                r1c = sml.tile([P, 1], f32, tag="r1c")
                nc.vector.tensor_tensor(r1c[:, :], cb[:, :], ps_k1[:, :], Alu.subtract)
                r1row = sml.tile([1, PH], f32, tag="r1row")
                r1B = sml.tile([128, PH], f32, tag="r1B")
                for hx in range(2):
                    psr = pp.tile([1, 384], f32, tag="ps2", bufs=1, name=f"psr_{hx}")
                    mm(psr[:, :], r1c[:, :], r12[:, ds(384 * hx, 384)], True, True)
                    nc.vector.tensor_copy(r1row[:, ds(384 * hx, 384)], psr[:, :])
                for hx in range(2):
                    psb = pp.tile([128, 384], f32, tag="ps2", bufs=1, name=f"psb_{hx}")
                    mm(psb[:, :], onesm[:, :], r1row[:, ds(384 * hx, 384)], True, True)
                    nc.vector.tensor_copy(r1B[:, ds(384 * hx, 384)], psb[:, :])
                return r1B

            # ---- prefetch next batch: X loads + LN1 stats before pass-2 ----
            if b + 1 < BL:
                cc_n, nc_n = emit_stats(b + 1, Xn1)
                nxt = (Xn1, cc_n, nc_n)

            # ---- pass-2: out[n, (q,h)] = sum_c Z[:, c, n]^T @ W[:, c, :] ----
            r1B = None
            for ni in range(KT):
                po = [pp.tile([128, 384], f32, tag="ps", name=f"po_{b}_{ni}_{i}") for i in range(2)]
                for kc in range(MC if "pass2" not in SKIP else 1):
                    for hx in range(2):
                        mm(po[hx][:, :], Z[:, kc, ds(ni * 128, 128)],
                           W[:, kc, ds(384 * hx, 384)], kc == 0,
                           (kc == MC - 1 or "pass2" in SKIP))
                if r1B is None:
                    r1B = emit_r1()
                if ni % 2 == 0:
                    stage4 = sp.tile([128, 2, P, H], bf16, tag="ostage")
                with nc.allow_low_precision(reason="out stored bf16"):
                    for hx in range(2):
                        nc.vector.tensor_tensor(
                            stage4[:, ni % 2, ds(6 * hx, 6), :],
                            po[hx].rearrange("t (p h) -> t p h", h=H),
                            r1B[:, ds(384 * hx, 384)].rearrange("t (p h) -> t p h", h=H),
                            Alu.add)
                if "out" not in SKIP and ni >= KT - 2:
                    eng = nc.scalar if ni % 2 == 0 else nc.gpsimd
                    eng.dma_start(
                        out=out_d[b][ni, :, :, :],
                        in_=stage4[:, ni % 2, :, :])
                elif "out" not in SKIP and ni % 2 == 1:
                    eng = nc.scalar if (ni // 2) % 2 == 0 else nc.gpsimd
                    eng.dma_start(
                        out=out_d[b][ds(ni - 1, 2), :, :, :].transpose([1, 0, 2, 3]),
                        in_=stage4[:, :, :, :])

    nc.compile()
    return nc


def _host_prep(inputs):
    import ml_dtypes
    bf = ml_dtypes.bfloat16
    e4 = ml_dtypes.float8_e4m3
    x = np.asarray(inputs["x"], dtype=np.float32)
    edge_index = np.asarray(inputs["edge_index"])
    g_w = np.asarray(inputs["g_norm_w"], dtype=np.float32)
    g_b = np.asarray(inputs["g_norm_b"], dtype=np.float32)
    t_w = np.asarray(inputs["t_norm_w"], dtype=np.float32)
    t_b = np.asarray(inputs["t_norm_b"], dtype=np.float32)
    conv_w = np.asarray(inputs["conv_w"], dtype=np.float32)
    conv_b = np.asarray(inputs["conv_b"], dtype=np.float32)

    # fast path requires LN affine params constant (true for this problem family)
    assert np.all(g_w == g_w.flat[0]) and np.all(t_w == t_w.flat[0]), \
        "non-constant LayerNorm weight not supported by this kernel"
    kg = float(g_w.flat[0])
    kt = float(t_w.flat[0])
    assert np.all(t_b == t_b.flat[0]), "non-constant t_norm_b not supported"
    kb = float(t_b.flat[0])

    src = edge_index[0].astype(np.int64)
    dst = edge_index[1].astype(np.int64)
    deg = np.zeros(N, np.float32)
    np.add.at(deg, dst, np.float32(1.0))
    dinv = np.where(deg > 0, 1.0 / np.sqrt(np.maximum(deg, 1.0)), 0.0).astype(np.float32)

    # keep only edges with nonzero weight (dinv[src] > 0; dst always has deg>=1)
    keep = dinv[src] > 0
    srck, dstk = src[keep], dst[keep]

    # S: integer edge counts, exact in fp8. Row = src, col = dst.
    Sf = np.zeros((N, N), np.float32)
    np.add.at(Sf, (srck, dstk), np.float32(1.0))
    s8 = np.ascontiguousarray(Sf.reshape(128, KP, 2, N)).astype(e4)

    # u1[dst] = sum_e dinv[src_e]; corr folded as (c*G + ncu*u1 [+v1]) * dd
    u1 = np.zeros(N, np.float32)
    np.add.at(u1, dstk, dinv[srck])
    ddr = (kg * dinv).astype(np.float32)
    u1t2 = np.ascontiguousarray(np.broadcast_to(u1, (128, N))).astype(bf)
    ddt2 = np.ascontiguousarray(np.broadcast_to(ddr, (128, N))).astype(bf)

    # v = A @ g_b (element-wise over h); v1 = v / dd  (guard dd == 0)
    has_v = bool(np.any(g_b != 0))
    if has_v:
        A = np.zeros((N, N), np.float32)
        A[dstk, srck] = 0.0
        np.add.at(A, (dstk, srck), (dinv[srck] * dinv[dstk]).astype(np.float32))
        v = A @ g_b          # [N, H]
        vt2 = np.empty((128, N), np.float32)
        vt2[:64] = v.T; vt2[64:] = v.T
        dd32 = ddt2.astype(np.float32)
        with np.errstate(divide="ignore", invalid="ignore"):
            v1t2 = np.where(dd32 != 0, vt2 / dd32, 0.0).astype(np.float32)
    else:
        v1t2 = np.zeros((1, 1), np.float32)  # unused

    # x scaled by dinv[src], split hi+lo fp8; plus raw bf16 for LN1 stats
    xb = np.ascontiguousarray(
        x.astype(bf).reshape(B, P, 128, KT, H).transpose(0, 2, 3, 1, 4))
    xs = x * dinv[None, None, :, None]
    xs = np.ascontiguousarray(
        xs.reshape(B, P, 128, KP, 2, H).transpose(0, 2, 3, 4, 1, 5))
    x8h = xs.astype(e4)
    x8l = (xs - x8h.astype(np.float32)).astype(e4)

    cwi = np.zeros((PH, PH), np.float32)
    for p in range(P):
        for q in range(P):
            w = conv_w[q, p] * kt
            idx = np.arange(H)
            cwi[p * H + idx, q * H + idx] = w
    cwi = cwi.astype(bf)

    r12 = np.zeros((P, PH), np.float32)
    for p in range(P):
        r12[p, p * H:(p + 1) * H] = 1.0
    bo = np.zeros((PH, P), np.float32)
    for p in range(P):
        bo[p * H:(p + 1) * H, p] = 1.0
    cwt = np.ascontiguousarray(conv_w.T * kt)
    cb = (conv_b + kb * conv_w.sum(axis=1)).astype(np.float32).reshape(P, 1)

    consts = {"s8": s8, "u1": u1t2, "dd": ddt2, "cwi": cwi,
              "r12": r12, "bo": bo, "cwt": cwt, "cb": cb}
    if has_v:
        consts["v1"] = v1t2
    return (xb, x8h, x8l), consts, has_v


def _unpack_out(arr):
    """[BL, KT(ni), 128, P, H] bf16 -> [BL, P, N, H] f32 with n = ni*128 + t."""
    return np.ascontiguousarray(
        arr.astype(np.float32).transpose(0, 3, 1, 2, 4).reshape(BL, P, N, H))


def kernel(**inputs):
    from concourse.bass_utils import run_bass_kernel_spmd

    (xb, x8h, x8l), consts, has_v = _host_prep(inputs)

    if ("nc", has_v) not in _CACHE:
        _CACHE[("nc", has_v)] = _build_program(has_v)
    nc = _CACHE[("nc", has_v)]

    in_maps = []
    for c in range(NCORES):
        sl = slice(c * BL, (c + 1) * BL)
        m = {"xb": np.ascontiguousarray(xb[sl]),
             "x8h": np.ascontiguousarray(x8h[sl]),
             "x8l": np.ascontiguousarray(x8l[sl])}
        m.update(consts)
        in_maps.append(m)

    res = run_bass_kernel_spmd(nc, in_maps, core_ids=list(range(NCORES)))
    out = np.empty((B, P, N, H), np.float32)
    for c in range(NCORES):
        out[c * BL:(c + 1) * BL] = _unpack_out(res.results[c]["out"])
    return out
